# revision 1
# baseline (speedup 1.0000x reference)
"""Trainium2 Bass kernel for nn_JointRelationModule (self-contained).

Math (per person p, all within one imgid group for the softmax):
    q = Wq x + bq ; k = Wk x + bk ; v = Wv x + bv          (1x1 conv over K=17)
    S_p = q_p k_p^T / 64                                   ([17,17] scores)
    attn = segment-softmax over the person dim (per imgid group, per (i,j))
    out = relu(attn_p @ v_p + x_p)

Key reformulation used on device: with G_p = x_p x_p^T (17x17 Gram),
    S_p = Wq G_p Wk^T / 64 (+ cheap rank-1 bias terms)
    attn_p @ v_p = (attn_p @ Wv) @ x_p (+ (attn_p @ bv) broadcast)
so the only O(p*K*hw) device work is: transpose x (PE), Gram (PE), and the
final (attn Wv) @ x matmul (PE, float32r) + residual/relu (DVE/ACT).

Sharding: data-parallel over persons, split at imgid group boundaries
(8 cores), weights replicated. Segment softmax runs fully on-device via
indicator-matrix matmuls (persons on partitions); the indicator is built on
the host from imgid (sharding metadata, not compute).
"""

import math
import sys

import numpy as np

K = 17
HW = 4096  # 64*64
P_TOTAL = 512
N_CORES = 8
NORM = 64.0
BD = 7          # persons per block-diagonal stack
BDK = BD * K    # 119
D_CH = 128      # transpose / gram chunk along hw dim
O_CH = 512      # output chunk along hw dim (one PSUM bank of f32)

_cache: dict = {}


def _ensure_path():
    try:
        import concourse.bass  # noqa: F401
    except ImportError:
        for p in ("/opt/trn_rl_repo", "/root/.axon_site/_ro/trn_rl_repo"):
            if p not in sys.path:
                sys.path.insert(0, p)
        import concourse.bass  # noqa: F401


def _build(P_pad: int, G_pad: int):
    """Builds + compiles the per-core SPMD Bass program."""
    _ensure_path()
    import concourse.bacc as bacc
    import concourse.mybir as mybir
    import concourse.tile as tile

    f32 = mybir.dt.float32
    bf16 = mybir.dt.bfloat16
    Exp = mybir.ActivationFunctionType.Exp
    Relu = mybir.ActivationFunctionType.Relu

    S = P_pad // BD
    assert P_pad % BD == 0 and P_pad <= 128 and G_pad <= 128
    n_dch = HW // D_CH   # 32
    n_och = HW // O_CH   # 8
    resident = S <= 10   # all of x stays in SBUF

    nc = bacc.Bacc(
        "TRN2",
        target_bir_lowering=False,
        debug=False,
        enable_asserts=False,
        num_devices=N_CORES,
    )

    x_d = nc.dram_tensor("x", [P_pad * K, HW], f32, kind="ExternalInput")
    wq_d = nc.dram_tensor("wq64t_bd", [BDK, BDK], f32, kind="ExternalInput")
    wk_d = nc.dram_tensor("wkt_bd", [BDK, BDK], f32, kind="ExternalInput")
    wv_d = nc.dram_tensor("wv_bd", [BDK, BDK], f32, kind="ExternalInput")
    i_d = nc.dram_tensor("i119", [BDK, BDK], f32, kind="ExternalInput")
    ind_d = nc.dram_tensor("ind", [P_pad, G_pad], f32, kind="ExternalInput")
    indt_d = nc.dram_tensor("indT", [G_pad, P_pad], f32, kind="ExternalInput")
    corr_d = nc.dram_tensor("corr", [P_pad, K * K], f32, kind="ExternalInput")
    bv_d = nc.dram_tensor("bv119", [BDK, 1], f32, kind="ExternalInput")
    y_d = nc.dram_tensor("y", [P_pad * K, HW], f32, kind="ExternalOutput")

    with tile.TileContext(nc) as tc:
        with (
            tc.tile_pool(name="xpool", bufs=1) as xpool,
            tc.tile_pool(name="cpool", bufs=1) as cpool,
            tc.tile_pool(name="wpool", bufs=2) as wpool,
            tc.tile_pool(name="fpool", bufs=1) as fpool,
            tc.tile_pool(name="opool", bufs=3) as opool,
            tc.tile_pool(name="pp", bufs=2, space="PSUM") as pp,
        ):
            # --- replicated constants ---
            wq_t = cpool.tile([BDK, BDK], f32, name="wq_t", tag="wq")
            wk_t = cpool.tile([BDK, BDK], f32, name="wk_t", tag="wk")
            wv_t = cpool.tile([BDK, BDK], f32, name="wv_t", tag="wv")
            id_t = cpool.tile([BDK, BDK], f32, name="id_t", tag="id")
            ind_t = cpool.tile([P_pad, G_pad], f32, name="ind_t", tag="ind")
            indt_t = cpool.tile([G_pad, P_pad], f32, name="indt_t", tag="indt")
            bv_t = cpool.tile([BDK, 1], f32, name="bv_t", tag="bv")
            nc.sync.dma_start(wq_t[:], wq_d.ap())
            nc.sync.dma_start(wk_t[:], wk_d.ap())
            nc.sync.dma_start(wv_t[:], wv_d.ap())
            nc.sync.dma_start(id_t[:], i_d.ap())
            nc.sync.dma_start(ind_t[:], ind_d.ap())
            nc.sync.dma_start(indt_t[:], indt_d.ap())
            nc.sync.dma_start(bv_t[:], bv_d.ap())

            e_flat = fpool.tile([P_pad, K * K], f32, name="e_flat", tag="e")
            corr_t = fpool.tile([P_pad, K * K], f32, name="corr_t", tag="corr")
            nc.sync.dma_start(corr_t[:], corr_d.ap())

            # --- phase A+B: per stack, gram -> scores^T -> extract ---
            x_tiles = []
            ncopy = 0
            for s in range(S):
                if resident:
                    xs = xpool.tile([BDK, HW], f32, name=f"xs{s}", tag=f"xs{s}")
                else:
                    xs = xpool.tile([BDK, HW], f32, name=f"xs{s}", tag="xs",
                                    bufs=3)
                # chunked load: spreads across DMA queues and lets the first
                # transposes start ~8x earlier than one monolithic 1.95MB DMA
                for lc in range(8):
                    lsl = slice(512 * lc, 512 * (lc + 1))
                    nc.sync.dma_start(
                        xs[:, lsl], x_d.ap()[BDK * s:BDK * (s + 1), lsl]
                    )
                x_tiles.append(xs)

                g_ps = pp.tile([BDK, BDK], f32, name=f"g{s}", tag="g", bufs=2)
                for dc in range(n_dch):
                    tp = pp.tile([D_CH, BDK], f32, name="tp", tag="tp", bufs=2)
                    nc.tensor.transpose(
                        tp[:], xs[:, D_CH * dc:D_CH * (dc + 1)], id_t[:]
                    )
                    xt_sb = wpool.tile([D_CH, BDK], f32, name="xt_sb", tag="xt")
                    # split PSUM->SBUF copies between DVE and ACT
                    if ncopy % 3 == 0:
                        nc.vector.tensor_copy(xt_sb[:], tp[:])
                    else:
                        nc.scalar.copy(xt_sb[:], tp[:])
                    ncopy += 1
                    nc.tensor.matmul(
                        g_ps[:], xt_sb[:], xt_sb[:],
                        start=(dc == 0), stop=(dc == n_dch - 1),
                    )

                # tiny chain: ST_stack = BD(Wk) @ (G @ BD(Wq^T/64))
                g_sb = wpool.tile([BDK, BDK], f32, name="g_sb", tag="g_sb")
                nc.vector.tensor_copy(g_sb[:], g_ps[:])
                m1_ps = pp.tile([BDK, BDK], f32, name="m1", tag="tiny", bufs=2)
                nc.tensor.matmul(m1_ps[:], g_sb[:], wq_t[:], start=True, stop=True)
                m1_sb = wpool.tile([BDK, BDK], f32, name="m1_sb", tag="m1_sb")
                nc.scalar.copy(m1_sb[:], m1_ps[:])
                st_ps = pp.tile([BDK, BDK], f32, name="st", tag="tiny", bufs=2)
                nc.tensor.matmul(st_ps[:], wk_t[:], m1_sb[:], start=True, stop=True)
                st_sb = wpool.tile([BDK, BDK], f32, name="st_sb", tag="st_sb")
                nc.vector.tensor_copy(st_sb[:], st_ps[:])
                for j in range(BD):
                    p = BD * s + j
                    nc.gpsimd.dma_start(
                        e_flat[p:p + 1, :],
                        st_sb[K * j:K * (j + 1), K * j:K * (j + 1)],
                    )

            # --- phase C: segment softmax over persons (on partitions) ---
            e_bias = fpool.tile([P_pad, K * K], f32, name="e_bias", tag="eb")
            nc.vector.tensor_add(e_bias[:], e_flat[:], corr_t[:])
            exp_flat = fpool.tile([P_pad, K * K], f32, name="exp_flat", tag="exp")
            nc.scalar.activation(exp_flat[:], e_bias[:], Exp)
            seg_ps = pp.tile([G_pad, K * K], f32, name="seg", tag="tiny", bufs=2)
            nc.tensor.matmul(seg_ps[:], ind_t[:], exp_flat[:], start=True, stop=True)
            seg_sb = fpool.tile([G_pad, K * K], f32, name="seg_sb", tag="seg")
            nc.vector.tensor_scalar_max(seg_sb[:], seg_ps[:], 1e-30)
            inv_sb = fpool.tile([G_pad, K * K], f32, name="inv_sb", tag="inv")
            nc.vector.reciprocal(inv_sb[:], seg_sb[:])
            invb_ps = pp.tile([P_pad, K * K], f32, name="invb", tag="tiny", bufs=2)
            nc.tensor.matmul(invb_ps[:], indt_t[:], inv_sb[:], start=True, stop=True)
            attn_flat = fpool.tile([P_pad, K * K], f32, name="attn_flat", tag="at")
            nc.vector.tensor_mul(attn_flat[:], exp_flat[:], invb_ps[:])

            # --- phase D: AT = BD(Wv^T attn^T); out = relu(AT.T @ x + x) ---
            for s in range(S):
                bdat = wpool.tile([BDK, BDK], f32, name="bdat", tag="bdat")
                nc.gpsimd.memset(bdat[:], 0.0)
                for j in range(BD):
                    p = BD * s + j
                    nc.gpsimd.dma_start(
                        bdat[K * j:K * (j + 1), K * j:K * (j + 1)],
                        attn_flat[p:p + 1, :],
                    )
                at_ps = pp.tile([BDK, BDK], f32, name="at", tag="tiny", bufs=2)
                nc.tensor.matmul(at_ps[:], wv_t[:], bdat[:], start=True, stop=True)
                at_sb = wpool.tile([BDK, BDK], bf16, name="at_sb", tag="at_sb")
                nc.scalar.copy(at_sb[:], at_ps[:])
                # attnv[17j+i] = sum_m attn^T[m,i] bv[m]  (v-bias broadcast term)
                av_ps = pp.tile([BDK, 1], f32, name="av", tag="tiny", bufs=2)
                nc.tensor.matmul(av_ps[:], bdat[:], bv_t[:], start=True, stop=True)
                av_sb = wpool.tile([BDK, 1], f32, name="av_sb", tag="av_sb")
                nc.vector.tensor_copy(av_sb[:], av_ps[:])

                for oc in range(n_och):
                    sl = slice(O_CH * oc, O_CH * (oc + 1))
                    if resident:
                        xr = x_tiles[s]
                        x_ap = xr[:, sl]
                    else:
                        xchunk = opool.tile([BDK, O_CH], f32, name="xchunk",
                                            tag="xc")
                        nc.sync.dma_start(
                            xchunk[:], x_d.ap()[BDK * s:BDK * (s + 1), sl]
                        )
                        x_ap = xchunk[:]
                    xbf = opool.tile([BDK, O_CH], bf16, name="xbf", tag="xbf")
                    nc.vector.tensor_copy(xbf[:], x_ap)
                    o_ps = pp.tile([BDK, O_CH], f32, name="o_ps", tag="ops", bufs=2)
                    nc.tensor.matmul(
                        o_ps[:], at_sb[:], xbf[:], start=True, stop=True,
                    )
                    sum_sb = opool.tile([BDK, O_CH], f32, name="sum_sb", tag="sum")
                    nc.vector.tensor_add(sum_sb[:], o_ps[:], x_ap)
                    res_sb = opool.tile([BDK, O_CH], f32, name="res_sb", tag="res")
                    nc.scalar.activation(res_sb[:], sum_sb[:], Relu,
                                         bias=av_sb[:, 0:1])
                    (nc.sync if oc % 2 == 0 else nc.gpsimd).dma_start(
                        y_d.ap()[BDK * s:BDK * (s + 1), sl], res_sb[:]
                    )

    nc.compile()
    return nc


def _get_compiled(P_pad: int, G_pad: int):
    key = (P_pad, G_pad)
    if key not in _cache:
        _cache[key] = _build(P_pad, G_pad)
    return _cache[key]


def _bd7(m: np.ndarray) -> np.ndarray:
    out = np.zeros((BDK, BDK), dtype=np.float32)
    for j in range(BD):
        out[K * j:K * (j + 1), K * j:K * (j + 1)] = m
    return out


def _plan(ids: np.ndarray):
    """Split persons into N_CORES contiguous chunks at imgid boundaries."""
    change = np.flatnonzero(np.diff(ids)) + 1
    allb = np.concatenate([[0], change, [P_TOTAL]]).astype(np.int64)
    bounds = [0]
    for ci in range(1, N_CORES):
        target = P_TOTAL * ci / N_CORES
        cand = allb[allb > bounds[-1]]
        if len(cand) == 0:
            bounds.append(bounds[-1])
        else:
            bounds.append(int(cand[np.argmin(np.abs(cand - target))]))
    bounds.append(P_TOTAL)
    sizes = np.diff(bounds)
    P_max = int(sizes.max())
    P_pad = max(BD, BD * math.ceil(P_max / BD))
    g_max = 0
    for ci in range(N_CORES):
        a, b = bounds[ci], bounds[ci + 1]
        g_max = max(g_max, len(np.unique(ids[a:b])))
    G_pad = max(4, 4 * math.ceil((g_max + 1) / 4))
    return bounds, P_pad, G_pad


def _prepare(inputs: dict):
    x = np.ascontiguousarray(
        np.asarray(inputs["kpt_feat"], dtype=np.float32).reshape(P_TOTAL, K, HW)
    )
    ids = np.asarray(inputs["imgid"]).astype(np.int64)
    Wq = np.asarray(inputs["Wq"], np.float32)
    Wk = np.asarray(inputs["Wk"], np.float32)
    Wv = np.asarray(inputs["Wv"], np.float32)
    bq = np.asarray(inputs["bq"], np.float32)
    bk = np.asarray(inputs["bk"], np.float32)
    bv = np.asarray(inputs["bv"], np.float32)

    bounds, P_pad, G_pad = _plan(ids)

    wq64t = _bd7((Wq.T / NORM).astype(np.float32))
    wkt = _bd7(Wk.T.astype(np.float32))
    wvb = _bd7(Wv.astype(np.float32))
    i119 = np.eye(BDK, dtype=np.float32)
    bv119 = np.tile(bv.reshape(K, 1), (BD, 1)).astype(np.float32)

    have_bias = bool(np.any(bq) or np.any(bk))
    if have_bias:
        xsum = x.sum(axis=2)                    # [P, K]
        qx = xsum @ Wq.T                        # [P, i]
        kx = xsum @ Wk.T                        # [P, m]
        corr_all = (
            bk[None, :, None] * qx[:, None, :]
            + bq[None, None, :] * kx[:, :, None]
            + HW * (bq[None, None, :] * bk[None, :, None])
        ) / NORM                                # [P, m, i]
        corr_all = corr_all.reshape(P_TOTAL, K * K).astype(np.float32)
    else:
        corr_all = np.zeros((P_TOTAL, K * K), dtype=np.float32)

    in_maps = []
    for ci in range(N_CORES):
        a, b = bounds[ci], bounds[ci + 1]
        pc = b - a
        xs = np.zeros((P_pad * K, HW), dtype=np.float32)
        if pc:
            xs[:pc * K] = x[a:b].reshape(pc * K, HW)
        corr = np.zeros((P_pad, K * K), dtype=np.float32)
        if pc:
            corr[:pc] = corr_all[a:b]
        ind = np.zeros((P_pad, G_pad), dtype=np.float32)
        if pc:
            lids = ids[a:b]
            _, lg = np.unique(lids, return_inverse=True)
            ind[np.arange(pc), lg] = 1.0
        ind[pc:, G_pad - 1] = 1.0
        in_maps.append({
            "x": xs,
            "wq64t_bd": wq64t,
            "wkt_bd": wkt,
            "wv_bd": wvb,
            "i119": i119,
            "ind": ind,
            "indT": np.ascontiguousarray(ind.T),
            "corr": corr,
            "bv119": bv119,
        })
    return in_maps, bounds, P_pad, G_pad


def _gather(results, bounds):
    out = np.empty((P_TOTAL, K, 64, 64), dtype=np.float32)
    for ci in range(N_CORES):
        a, b = bounds[ci], bounds[ci + 1]
        pc = b - a
        if pc:
            y = results[ci]["y"][:pc * K].reshape(pc, K, 64, 64)
            out[a:b] = y
    return out


def _run(inputs: dict, trace: bool = False):
    _ensure_path()
    from concourse.bass_utils import run_bass_kernel_spmd

    in_maps, bounds, P_pad, G_pad = _prepare(inputs)
    nc = _get_compiled(P_pad, G_pad)
    res = run_bass_kernel_spmd(nc, in_maps, list(range(N_CORES)), trace=trace)
    return _gather(res.results, bounds), res


def kernel(**inputs) -> np.ndarray:
    out, _ = _run(inputs, trace=False)
    return out



# revision 22
# speedup vs baseline: 2.0625x; 2.0625x over previous
"""Trainium2 Bass kernel for nn_JointRelationModule (self-contained).

Math (per person p; softmax is segment-softmax over persons within an imgid
group, elementwise over the (K,K) score entries):
    q = Wq x + bq ; k = Wk x + bk ; v = Wv x + bv      (1x1 conv over K=17)
    S_p = q_p k_p^T / 64
    attn = segment-softmax over persons
    out = relu(attn_p @ v_p + x_p)

Device formulation (all heavy ops bf16 on the PE, block-column layouts):
  - Stack BD=7 persons as [119, hw]. Per stack: G = x x^T via PE transpose +
    accumulating matmuls (bf16, f32 PSUM).
  - scores^T in block-column layout [119, 17]: st = BD(Wk^T)^T @ (G_masked^T
    @ (Wq^T/64 tiled)) using a block-diag mask on G so a plain [119,17]
    matmul chain yields per-person scores with no cross-person terms.
  - Segment softmax via per-stack selector matmuls into group-slot tiles
    (persons land on partitions), reciprocal, selector-transpose broadcast
    back; all partition-aligned, no SBUF gather/scatter DMAs.
  - Output: B = blockdiag((attn Wv)^T) built by 7 per-person matmuls with a
    shared augmented stationary (Wv | bv); residual folded in by adding I and
    an all-ones row on x, so out chunk = relu((B+I)^T @ x_aug) in one matmul
    + one elementwise op. Stored bf16 and upcast on the host.

Sharding: data-parallel over persons at imgid group boundaries (8 cores),
weights replicated. Input x is host-cast to bf16 (halves load traffic);
output returned bf16 and upcast (halves store traffic). Tolerance is 2e-2;
measured end-to-end error ~5e-3.
"""

import math
import sys

import numpy as np

K = 17
HW = 4096  # 64*64
P_TOTAL = 512
N_CORES = 8
NORM = 64.0
BD = 7          # persons per stack
BDK = BD * K    # 119
O_CH = 512      # output chunk cols (one PSUM bank of f32)

_cache: dict = {}


def _ensure_path():
    try:
        import concourse.bass  # noqa: F401
    except ImportError:
        for p in ("/opt/trn_rl_repo", "/root/.axon_site/_ro/trn_rl_repo"):
            if p not in sys.path:
                sys.path.insert(0, p)
        import concourse.bass  # noqa: F401


def _build(P_pad: int, T: int, have_bias: bool):
    """Builds + compiles the per-core SPMD Bass program."""
    _ensure_path()
    import concourse.bacc as bacc
    import concourse.mybir as mybir
    import concourse.tile as tile

    f32 = mybir.dt.float32
    bf16 = mybir.dt.bfloat16
    Exp = mybir.ActivationFunctionType.Exp
    Relu = mybir.ActivationFunctionType.Relu

    S = P_pad // BD
    assert P_pad % BD == 0
    n_grp = HW // 512   # 8 groups of 4x128 cols per stack

    nc = bacc.Bacc(
        "TRN2",
        target_bir_lowering=False,
        debug=False,
        enable_asserts=False,
        num_devices=N_CORES,
    )

    x_d = nc.dram_tensor("x", [S * (BDK + 1), HW], bf16, kind="ExternalInput")
    wq_d = nc.dram_tensor("wq_col", [BDK, K], f32, kind="ExternalInput")
    wk_d = nc.dram_tensor("wkt_bd", [BDK, BDK], f32, kind="ExternalInput")
    wv_d = nc.dram_tensor("wv_aug", [BDK, BDK + 1], bf16, kind="ExternalInput")
    id_d = nc.dram_tensor("id119", [BDK, BDK], bf16, kind="ExternalInput")
    ia_d = nc.dram_tensor("iaug", [BDK + 1, BDK], f32, kind="ExternalInput")
    mk_d = nc.dram_tensor("bdmask", [BDK, BDK], f32, kind="ExternalInput")
    mkb_d = nc.dram_tensor("bdmaskb", [BDK, BDK], bf16, kind="ExternalInput")
    sel_d = nc.dram_tensor("sel", [BDK, S * T * BDK], f32, kind="ExternalInput")
    selt_d = nc.dram_tensor("selT", [BDK, S * T * BDK], f32,
                            kind="ExternalInput")
    if have_bias:
        corr_d = nc.dram_tensor("corr_col", [BDK, K * S], f32,
                                kind="ExternalInput")
    y_d = nc.dram_tensor("y", [P_pad * K, HW], bf16, kind="ExternalOutput")

    with tile.TileContext(nc) as tc:
        with (
            tc.tile_pool(name="xpool", bufs=1) as xpool,
            tc.tile_pool(name="cpool", bufs=1) as cpool,
            tc.tile_pool(name="wpool", bufs=2) as wpool,
            tc.tile_pool(name="opool", bufs=2) as opool,
            tc.tile_pool(name="pp", bufs=2, space="PSUM") as pp,
        ):
            # --- x loads first (critical path), then constants ---
            x_tiles = []
            for s in range(S):
                xs = xpool.tile([BDK + 1, HW], bf16, name=f"xs{s}",
                                tag=f"xs{s}")
                (nc.sync if s % 2 == 0 else nc.gpsimd).dma_start(
                    xs[:], x_d.ap()[(BDK + 1) * s:(BDK + 1) * (s + 1), :]
                )
                x_tiles.append(xs)

            id_t = cpool.tile([BDK, BDK], bf16, name="id_t", tag="id")
            mk_t = cpool.tile([BDK, BDK], f32, name="mk_t", tag="mk")
            wq_t = cpool.tile([BDK, K], f32, name="wq_t", tag="wq")
            wk_t = cpool.tile([BDK, BDK], f32, name="wk_t", tag="wk")
            nc.sync.dma_start(id_t[:], id_d.ap())
            nc.gpsimd.dma_start(mk_t[:], mk_d.ap())
            nc.sync.dma_start(wq_t[:], wq_d.ap())
            nc.gpsimd.dma_start(wk_t[:], wk_d.ap())
            wv_t = cpool.tile([BDK, BDK + 1], bf16, name="wv_t", tag="wv")
            ia_t = cpool.tile([BDK + 1, BDK], f32, name="ia_t", tag="ia")
            sel_t = cpool.tile([BDK, S * T * BDK], f32, name="sel_t",
                               tag="sel")
            selt_t = cpool.tile([BDK, S * T * BDK], f32, name="selt_t",
                                tag="selt")
            mkb_t = cpool.tile([BDK, BDK], bf16, name="mkb_t", tag="mkb")
            nc.sync.dma_start(wv_t[:], wv_d.ap())
            nc.gpsimd.dma_start(ia_t[:], ia_d.ap())
            nc.sync.dma_start(mkb_t[:], mkb_d.ap())
            nc.sync.dma_start(sel_t[:], sel_d.ap())
            nc.gpsimd.dma_start(selt_t[:], selt_d.ap())
            if have_bias:
                corr_t = cpool.tile([BDK, K * S], f32, name="corr_t",
                                    tag="corr")
                nc.sync.dma_start(corr_t[:], corr_d.ap())

            exp_all = cpool.tile([BDK, K * S], f32, name="exp_all", tag="exp")
            inv_t = cpool.tile([BDK, K * T], f32, name="inv_t", tag="inv")

            # --- phase A: per stack, transpose -> gram -> scores^T -> exp ---
            # PSUM tags (8 banks total, bank-granular per buffer):
            #   big  = transpose staging (A) / output chunks (D), 4 bufs
            #   gsb  = gram (A) / segment sums (C) / B matrix (D), 2 bufs
            #   tiny = m1 / st / invb, 2 bufs
            G_CH = 1024          # x cols per transpose group (8 chunks)
            n_grp = HW // G_CH   # 4 groups per stack
            ncopy = 0
            g_tiles = {}
            pend = []  # (s, gi, xt) with transposes+copy emitted, gram not

            TC = BDK + 1  # 120: per-chunk col stride, 4B-aligned in PSUM

            def emit_transposes(s, gi):
                tp = pp.tile([128, 8 * TC], bf16, name="tp", tag="big",
                             bufs=4)
                for c8 in range(8):
                    col = G_CH * gi + 128 * c8
                    nc.tensor.transpose(
                        tp[:, TC * c8:TC * c8 + BDK],
                        x_tiles[s][0:BDK, col:col + 128], id_t[:],
                    )
                xt = wpool.tile([128, 8 * TC], bf16, name="xt", tag="xt",
                                bufs=4)
                if ncopy % 2 == 0:
                    nc.vector.tensor_copy(xt[:], tp[:])
                else:
                    nc.scalar.copy(xt[:], tp[:])
                return xt

            def emit_gram(s, gi, xt):
                if s not in g_tiles:
                    g_tiles[s] = pp.tile([BDK + 1, BDK], f32, name=f"g{s}",
                                         tag="gsb", bufs=2)
                g_ps = g_tiles[s]
                for c8 in range(8):
                    nc.tensor.matmul(
                        g_ps[0:BDK, :], xt[:, TC * c8:TC * c8 + BDK],
                        xt[:, TC * c8:TC * c8 + BDK],
                        start=(gi == 0 and c8 == 0),
                        stop=(gi == n_grp - 1 and c8 == 7),
                    )
                if gi == n_grp - 1:
                    emit_tiny_chain(s)

            def emit_tiny_chain(s):
                # g -> masked g -> m1 -> scores^T -> exp
                g_sb = wpool.tile([BDK, BDK], f32, name="g_sb", tag="g_sb",
                                  bufs=2)
                nc.vector.tensor_mul(g_sb[:], g_tiles[s][0:BDK, :], mk_t[:])
                m1_ps = pp.tile([BDK, K], f32, name="m1", tag="tiny", bufs=2)
                nc.tensor.matmul(m1_ps[:], g_sb[:], wq_t[:], start=True,
                                 stop=True)
                m1_sb = wpool.tile([BDK, K], f32, name="m1_sb", tag="m1_sb",
                                   bufs=2)
                nc.scalar.copy(m1_sb[:], m1_ps[:])
                st_ps = pp.tile([BDK, K], f32, name="st", tag="tiny", bufs=2)
                nc.tensor.matmul(st_ps[:], wk_t[:], m1_sb[:], start=True,
                                 stop=True)
                esl = slice(K * s, K * (s + 1))
                if have_bias:
                    eb_sb = wpool.tile([BDK, K], f32, name="eb_sb", tag="eb")
                    nc.vector.tensor_add(eb_sb[:], st_ps[:], corr_t[:, esl])
                    nc.scalar.activation(exp_all[:, esl], eb_sb[:], Exp)
                else:
                    nc.scalar.activation(exp_all[:, esl], st_ps[:], Exp)

            for s in range(S):
                for gi in range(n_grp):
                    pend.append((s, gi, emit_transposes(s, gi)))
                    ncopy += 1
                    if len(pend) > 2:
                        ps, pgi, xt = pend.pop(0)
                        emit_gram(ps, pgi, xt)
            for ps, pgi, xt in pend:
                emit_gram(ps, pgi, xt)

            # --- phase C: segment sums -> reciprocal (group slots on rows) ---
            seg_tiles = []
            for t in range(T):
                seg_ps = pp.tile([BDK + 1, BDK], f32, name=f"seg{t}",
                                 tag="gsb", bufs=2)
                for s in range(S):
                    nc.tensor.matmul(
                        seg_ps[0:BDK, 0:K],
                        sel_t[:, BDK * (s * T + t):BDK * (s * T + t + 1)],
                        exp_all[:, K * s:K * (s + 1)],
                        start=(s == 0), stop=(s == S - 1),
                    )
                seg_tiles.append(seg_ps)
            for t in range(T):
                seg_sb = wpool.tile([BDK, K], f32, name="seg_sb", tag="seg_sb")
                nc.vector.tensor_scalar_max(seg_sb[:], seg_tiles[t][0:BDK, 0:K],
                                            1e-30)
                nc.vector.reciprocal(inv_t[:, K * t:K * (t + 1)], seg_sb[:])

            # --- phase D: broadcast-back, B matrix, out = relu(B_aug^T x) ---
            nrelu = 0
            for s in range(S):
                xs = x_tiles[s]
                invb_ps = pp.tile([BDK, K], f32, name="invb", tag="tiny",
                                  bufs=2)
                for t in range(T):
                    nc.tensor.matmul(
                        invb_ps[:],
                        selt_t[:, BDK * (s * T + t):BDK * (s * T + t + 1)],
                        inv_t[:, K * t:K * (t + 1)],
                        start=(t == 0), stop=(t == T - 1),
                    )
                attn_sb = wpool.tile([BDK, K], bf16, name="attn_sb",
                                     tag="attn_c", bufs=2)
                nc.vector.tensor_mul(attn_sb[:], exp_all[:, K * s:K * (s + 1)],
                                     invb_ps[:])
                attn_bd = wpool.tile([BDK, BDK], bf16, name="attn_bd",
                                     tag="attn", bufs=2)
                for j in range(BD):
                    jsl = slice(K * j, K * (j + 1))
                    eng = nc.vector if j % 2 == 0 else nc.gpsimd
                    eng.tensor_mul(attn_bd[:, jsl], attn_sb[:],
                                   mkb_t[:, jsl])
                b_ps = pp.tile([BDK + 1, BDK], f32, name="b_ps", tag="gsb",
                               bufs=2)
                nc.tensor.matmul(b_ps[:], wv_t[:], attn_bd[:], start=True,
                                 stop=True)
                b_sb = wpool.tile([BDK + 1, BDK], bf16, name="b_sb", tag="B",
                                  bufs=2)
                nc.vector.tensor_add(b_sb[:], b_ps[:], ia_t[:])

                osb = opool.tile([BDK, HW], bf16, name="osb", tag="osb",
                                 bufs=2)
                for oc in range(HW // O_CH):
                    sl = slice(O_CH * oc, O_CH * (oc + 1))
                    o_ps = pp.tile([BDK, O_CH], f32, name="o_ps", tag="big",
                                   bufs=4)
                    nc.tensor.matmul(o_ps[:], b_sb[:], xs[:, sl], start=True,
                                     stop=True)
                    if nrelu % 2 == 0:
                        nc.scalar.activation(osb[:, sl], o_ps[:], Relu)
                    else:
                        nc.vector.tensor_scalar_max(osb[:, sl], o_ps[:], 0.0)
                    nrelu += 1
                (nc.sync if s % 2 == 0 else nc.gpsimd).dma_start(
                    y_d.ap()[BDK * s:BDK * (s + 1), :], osb[:]
                )

    nc.compile()
    return nc


def _get_compiled(P_pad: int, T: int, have_bias: bool):
    key = (P_pad, T, have_bias)
    if key not in _cache:
        _cache[key] = _build(P_pad, T, have_bias)
    return _cache[key]


def _bd7(m: np.ndarray) -> np.ndarray:
    out = np.zeros((BDK, BDK), dtype=np.float32)
    for j in range(BD):
        out[K * j:K * (j + 1), K * j:K * (j + 1)] = m
    return out


def _plan(ids: np.ndarray):
    """Split persons into N_CORES contiguous chunks at imgid boundaries."""
    change = np.flatnonzero(np.diff(ids)) + 1
    allb = np.concatenate([[0], change, [P_TOTAL]]).astype(np.int64)
    bounds = [0]
    for ci in range(1, N_CORES):
        target = P_TOTAL * ci / N_CORES
        cand = allb[allb > bounds[-1]]
        if len(cand) == 0:
            bounds.append(bounds[-1])
        else:
            bounds.append(int(cand[np.argmin(np.abs(cand - target))]))
    bounds.append(P_TOTAL)
    sizes = np.diff(bounds)
    P_max = int(sizes.max())
    P_pad = max(BD, BD * math.ceil(P_max / BD))
    ng_max = 1
    for ci in range(N_CORES):
        a, b = bounds[ci], bounds[ci + 1]
        ng_max = max(ng_max, len(np.unique(ids[a:b])) + 1)
    T = math.ceil(ng_max / BD)
    return bounds, P_pad, T


def _prepare(inputs: dict):
    import ml_dtypes
    nbf16 = ml_dtypes.bfloat16

    x = np.asarray(inputs["kpt_feat"], dtype=np.float32).reshape(
        P_TOTAL, K, HW)
    ids = np.asarray(inputs["imgid"]).astype(np.int64)
    Wq = np.asarray(inputs["Wq"], np.float32)
    Wk = np.asarray(inputs["Wk"], np.float32)
    Wv = np.asarray(inputs["Wv"], np.float32)
    bq = np.asarray(inputs["bq"], np.float32)
    bk = np.asarray(inputs["bk"], np.float32)
    bv = np.asarray(inputs["bv"], np.float32)

    bounds, P_pad, T = _plan(ids)
    S = P_pad // BD

    wq_col = np.zeros((BDK, K), np.float32)
    for j in range(BD):
        wq_col[K * j:K * (j + 1), :] = Wq.T / NORM
    wkt_bd = _bd7(Wk.T.astype(np.float32))
    wv_aug = np.zeros((BDK, BDK + 1), np.float32)
    wv_aug[:, :BDK] = _bd7(Wv)
    for j in range(BD):
        wv_aug[K * j:K * (j + 1), BDK] = bv
    wv_aug = wv_aug.astype(nbf16)
    id119 = np.eye(BDK, dtype=np.float32).astype(nbf16)
    iaug = np.zeros((BDK + 1, BDK), np.float32)
    iaug[:BDK, :BDK] = np.eye(BDK, dtype=np.float32)
    bdmask = _bd7(np.ones((K, K), np.float32))

    have_bias = bool(np.any(bq) or np.any(bk))
    if have_bias:
        xsum = x.sum(axis=2)
        qx = xsum @ Wq.T
        kx = xsum @ Wk.T
        corr_all = (bk[None, :, None] * qx[:, None, :]
                    + bq[None, None, :] * kx[:, :, None]
                    + HW * (bq[None, None, :] * bk[None, :, None])) / NORM
        corr_all = corr_all.astype(np.float32)  # [P, m, i]
    else:
        corr_all = None

    xb = x.astype(nbf16)

    in_maps = []
    for ci in range(N_CORES):
        a, b = bounds[ci], bounds[ci + 1]
        pc = b - a
        xs = np.zeros((S * (BDK + 1), HW), dtype=nbf16)
        xs[BDK::BDK + 1] = 1.0  # per-stack all-ones row (residual/bias fold)
        for s in range(S):
            lo, hi = BD * s, min(BD * (s + 1), pc)
            if hi > lo:
                xs[(BDK + 1) * s:(BDK + 1) * s + (hi - lo) * K] = \
                    xb[a + lo:a + hi].reshape((hi - lo) * K, HW)
        # group slots: dummy slot (last) for padding persons
        slots = np.full((P_pad,), 7 * T - 1, np.int64)
        if pc:
            _, lg = np.unique(ids[a:b], return_inverse=True)
            slots[:pc] = lg
        sel = np.zeros((S, T, BDK, BDK), np.float32)
        eye = np.eye(K, dtype=np.float32)
        for s in range(S):
            for j in range(BD):
                g = slots[BD * s + j]
                t, lgi = divmod(g, BD)
                sel[s, t, K * j:K * (j + 1), K * lgi:K * (lgi + 1)] = eye
        # pack as [119, (s*T+t)*119 + c] for a single 2D DMA
        sel_pack = sel.reshape(S * T, BDK, BDK).transpose(1, 0, 2).reshape(
            BDK, S * T * BDK)
        selt_pack = sel.reshape(S * T, BDK, BDK).transpose(2, 0, 1).reshape(
            BDK, S * T * BDK)
        m = {
            "x": xs,
            "wq_col": wq_col,
            "wkt_bd": wkt_bd,
            "wv_aug": wv_aug,
            "id119": id119,
            "iaug": iaug,
            "bdmask": bdmask,
            "bdmaskb": bdmask.astype(nbf16),
            "sel": np.ascontiguousarray(sel_pack),
            "selT": np.ascontiguousarray(selt_pack),
        }
        if have_bias:
            corr_col = np.zeros((BDK, K * S), np.float32)
            if pc:
                cpad = np.zeros((P_pad, K, K), np.float32)
                cpad[:pc] = corr_all[a:b]
                for s in range(S):
                    for j in range(BD):
                        corr_col[K * j:K * (j + 1), K * s:K * (s + 1)] = \
                            cpad[BD * s + j]
            m["corr_col"] = corr_col
        in_maps.append(m)
    return in_maps, bounds, P_pad, T, have_bias


def _gather(results, bounds, P_pad):
    out = np.empty((P_TOTAL, K, 64, 64), dtype=np.float32)
    for ci in range(N_CORES):
        a, b = bounds[ci], bounds[ci + 1]
        pc = b - a
        if pc:
            y = np.asarray(results[ci]["y"][:pc * K], dtype=np.float32)
            out[a:b] = y.reshape(pc, K, 64, 64)
    return out


def _run(inputs: dict, trace: bool = False):
    _ensure_path()
    from concourse.bass_utils import run_bass_kernel_spmd

    in_maps, bounds, P_pad, T, have_bias = _prepare(inputs)
    nc = _get_compiled(P_pad, T, have_bias)
    res = run_bass_kernel_spmd(nc, in_maps, list(range(N_CORES)), trace=trace)
    return _gather(res.results, bounds, P_pad), res


def kernel(**inputs) -> np.ndarray:
    out, _ = _run(inputs, trace=False)
    return out


# revision 26
# speedup vs baseline: 2.1899x; 1.0618x over previous
"""Trainium2 Bass kernel for nn_JointRelationModule (self-contained).

Math (per person p; softmax is segment-softmax over persons within an imgid
group, elementwise over the (K,K) score entries):
    q = Wq x + bq ; k = Wk x + bk ; v = Wv x + bv      (1x1 conv over K=17)
    S_p = q_p k_p^T / 64
    attn = segment-softmax over persons
    out = relu(attn_p @ v_p + x_p)

Device formulation (all heavy ops bf16 on the PE, block-column layouts):
  - Stack BD=7 persons as [119, hw]. Per stack: G = x x^T via PE transpose +
    accumulating matmuls (bf16, f32 PSUM).
  - scores^T in block-column layout [119, 17]: st = BD(Wk^T)^T @ (G_masked^T
    @ (Wq^T/64 tiled)) using a block-diag mask on G so a plain [119,17]
    matmul chain yields per-person scores with no cross-person terms.
  - Segment softmax via per-stack selector matmuls into group-slot tiles
    (persons land on partitions), reciprocal, selector-transpose broadcast
    back; all partition-aligned, no SBUF gather/scatter DMAs.
  - Output: B = blockdiag((attn Wv)^T) built by 7 per-person matmuls with a
    shared augmented stationary (Wv | bv); residual folded in by adding I and
    an all-ones row on x, so out chunk = relu((B+I)^T @ x_aug) in one matmul
    + one elementwise op. Stored bf16 and upcast on the host.

Sharding: data-parallel over persons at imgid group boundaries (8 cores),
weights replicated. Input x is host-cast to bf16 (halves load traffic);
output returned bf16 and upcast (halves store traffic). Tolerance is 2e-2;
measured end-to-end error ~5e-3.
"""

import math
import sys

import numpy as np

K = 17
HW = 4096  # 64*64
P_TOTAL = 512
N_CORES = 8
NORM = 64.0
BD = 7          # persons per stack
BDK = BD * K    # 119
O_CH = 512      # output chunk cols (one PSUM bank of f32)

_cache: dict = {}


def _ensure_path():
    try:
        import concourse.bass  # noqa: F401
    except ImportError:
        for p in ("/opt/trn_rl_repo", "/root/.axon_site/_ro/trn_rl_repo"):
            if p not in sys.path:
                sys.path.insert(0, p)
        import concourse.bass  # noqa: F401


def _build(P_pad: int, T: int, have_bias: bool):
    """Builds + compiles the per-core SPMD Bass program."""
    _ensure_path()
    import concourse.bacc as bacc
    import concourse.mybir as mybir
    import concourse.tile as tile

    f32 = mybir.dt.float32
    bf16 = mybir.dt.bfloat16
    Exp = mybir.ActivationFunctionType.Exp
    Relu = mybir.ActivationFunctionType.Relu

    S = P_pad // BD
    assert P_pad % BD == 0
    n_grp = HW // 512   # 8 groups of 4x128 cols per stack

    nc = bacc.Bacc(
        "TRN2",
        target_bir_lowering=False,
        debug=False,
        enable_asserts=False,
        num_devices=N_CORES,
    )

    x_d = nc.dram_tensor("x", [S * (BDK + 1), HW], bf16, kind="ExternalInput")
    wq_d = nc.dram_tensor("wq_col", [BDK, K], f32, kind="ExternalInput")
    wk_d = nc.dram_tensor("wkt_bd", [BDK, BDK], f32, kind="ExternalInput")
    wv_d = nc.dram_tensor("wv_aug", [BDK, BDK + 1], bf16, kind="ExternalInput")
    id_d = nc.dram_tensor("id119", [BDK, BDK], bf16, kind="ExternalInput")
    ia_d = nc.dram_tensor("iaug", [BDK + 1, BDK], f32, kind="ExternalInput")
    mk_d = nc.dram_tensor("bdmask", [BDK, BDK], f32, kind="ExternalInput")
    mkb_d = nc.dram_tensor("bdmaskb", [BDK, BDK], bf16, kind="ExternalInput")
    sel_d = nc.dram_tensor("sel", [BDK, S * T * BDK], f32, kind="ExternalInput")
    selt_d = nc.dram_tensor("selT", [BDK, S * T * BDK], f32,
                            kind="ExternalInput")
    if have_bias:
        corr_d = nc.dram_tensor("corr_col", [BDK, K * S], f32,
                                kind="ExternalInput")
    y_d = nc.dram_tensor("y", [P_pad * K, HW], bf16, kind="ExternalOutput")

    with tile.TileContext(nc) as tc:
        with (
            tc.tile_pool(name="xpool", bufs=1) as xpool,
            tc.tile_pool(name="cpool", bufs=1) as cpool,
            tc.tile_pool(name="wpool", bufs=2) as wpool,
            tc.tile_pool(name="opool", bufs=2) as opool,
            tc.tile_pool(name="pp", bufs=2, space="PSUM") as pp,
        ):
            # --- tiny phase-A constants first (id_t gates every transpose),
            # then x loads spread over 4 issue queues, then bulkier consts ---
            id_t = cpool.tile([BDK, BDK], bf16, name="id_t", tag="id")
            mk_t = cpool.tile([BDK, BDK], f32, name="mk_t", tag="mk")
            wq_t = cpool.tile([BDK, K], f32, name="wq_t", tag="wq")
            wk_t = cpool.tile([BDK, BDK], f32, name="wk_t", tag="wk")
            nc.sync.dma_start(id_t[:], id_d.ap())
            nc.gpsimd.dma_start(mk_t[:], mk_d.ap())
            nc.sync.dma_start(wq_t[:], wq_d.ap())
            nc.gpsimd.dma_start(wk_t[:], wk_d.ap())

            queues = (nc.sync, nc.gpsimd, nc.scalar)
            x_tiles = []
            for s in range(S):
                xs = xpool.tile([BDK + 1, HW], bf16, name=f"xs{s}",
                                tag=f"xs{s}")
                queues[s % 3].dma_start(
                    xs[:], x_d.ap()[(BDK + 1) * s:(BDK + 1) * (s + 1), :]
                )
                x_tiles.append(xs)

            wv_t = cpool.tile([BDK, BDK + 1], bf16, name="wv_t", tag="wv")
            ia_t = cpool.tile([BDK + 1, BDK], f32, name="ia_t", tag="ia")
            sel_t = cpool.tile([BDK, S * T * BDK], f32, name="sel_t",
                               tag="sel")
            selt_t = cpool.tile([BDK, S * T * BDK], f32, name="selt_t",
                                tag="selt")
            mkb_t = cpool.tile([BDK, BDK], bf16, name="mkb_t", tag="mkb")
            nc.sync.dma_start(wv_t[:], wv_d.ap())
            nc.gpsimd.dma_start(ia_t[:], ia_d.ap())
            nc.sync.dma_start(mkb_t[:], mkb_d.ap())
            nc.sync.dma_start(sel_t[:], sel_d.ap())
            nc.gpsimd.dma_start(selt_t[:], selt_d.ap())
            if have_bias:
                corr_t = cpool.tile([BDK, K * S], f32, name="corr_t",
                                    tag="corr")
                nc.sync.dma_start(corr_t[:], corr_d.ap())

            exp_all = cpool.tile([BDK, K * S], f32, name="exp_all", tag="exp")
            inv_t = cpool.tile([BDK, K * T], f32, name="inv_t", tag="inv")

            # --- phase A: per stack, transpose -> gram -> scores^T -> exp ---
            # PSUM tags (8 banks total, bank-granular per buffer):
            #   big  = transpose staging (A) / output chunks (D), 4 bufs
            #   gsb  = gram (A) / segment sums (C) / B matrix (D), 2 bufs
            #   tiny = m1 / st / invb, 2 bufs
            G_CH = 1024          # x cols per transpose group (8 chunks)
            n_grp = HW // G_CH   # 4 groups per stack
            ncopy = 0
            g_tiles = {}
            pend = []  # (s, gi, xt) with transposes+copy emitted, gram not

            TC = BDK + 1  # 120: per-chunk col stride, 4B-aligned in PSUM

            def emit_transposes(s, gi):
                tp = pp.tile([128, 8 * TC], bf16, name="tp", tag="big",
                             bufs=4)
                for c8 in range(8):
                    col = G_CH * gi + 128 * c8
                    nc.tensor.transpose(
                        tp[:, TC * c8:TC * c8 + BDK],
                        x_tiles[s][0:BDK, col:col + 128], id_t[:],
                    )
                xt = wpool.tile([128, 8 * TC], bf16, name="xt", tag="xt",
                                bufs=4)
                if ncopy % 2 == 0:
                    nc.vector.tensor_copy(xt[:], tp[:])
                else:
                    nc.scalar.copy(xt[:], tp[:])
                return xt

            def emit_gram(s, gi, xt):
                if s not in g_tiles:
                    g_tiles[s] = pp.tile([BDK + 1, BDK], f32, name=f"g{s}",
                                         tag="gsb", bufs=2)
                g_ps = g_tiles[s]
                for c8 in range(8):
                    nc.tensor.matmul(
                        g_ps[0:BDK, :], xt[:, TC * c8:TC * c8 + BDK],
                        xt[:, TC * c8:TC * c8 + BDK],
                        start=(gi == 0 and c8 == 0),
                        stop=(gi == n_grp - 1 and c8 == 7),
                    )
                if gi == n_grp - 1:
                    emit_tiny_chain(s)

            def emit_tiny_chain(s):
                # g -> masked g -> m1 -> scores^T -> exp
                g_sb = wpool.tile([BDK, BDK], f32, name="g_sb", tag="g_sb",
                                  bufs=2)
                nc.vector.tensor_mul(g_sb[:], g_tiles[s][0:BDK, :], mk_t[:])
                m1_ps = pp.tile([BDK, K], f32, name="m1", tag="tiny", bufs=2)
                nc.tensor.matmul(m1_ps[:], g_sb[:], wq_t[:], start=True,
                                 stop=True)
                m1_sb = wpool.tile([BDK, K], f32, name="m1_sb", tag="m1_sb",
                                   bufs=2)
                nc.scalar.copy(m1_sb[:], m1_ps[:])
                st_ps = pp.tile([BDK, K], f32, name="st", tag="tiny", bufs=2)
                nc.tensor.matmul(st_ps[:], wk_t[:], m1_sb[:], start=True,
                                 stop=True)
                esl = slice(K * s, K * (s + 1))
                if have_bias:
                    eb_sb = wpool.tile([BDK, K], f32, name="eb_sb", tag="eb")
                    nc.vector.tensor_add(eb_sb[:], st_ps[:], corr_t[:, esl])
                    nc.scalar.activation(exp_all[:, esl], eb_sb[:], Exp)
                else:
                    nc.scalar.activation(exp_all[:, esl], st_ps[:], Exp)

            for s in range(S):
                for gi in range(n_grp):
                    pend.append((s, gi, emit_transposes(s, gi)))
                    ncopy += 1
                    if len(pend) > 2:
                        ps, pgi, xt = pend.pop(0)
                        emit_gram(ps, pgi, xt)
            for ps, pgi, xt in pend:
                emit_gram(ps, pgi, xt)

            # --- phase C: segment sums -> reciprocal (group slots on rows) ---
            seg_tiles = []
            for t in range(T):
                seg_ps = pp.tile([BDK + 1, BDK], f32, name=f"seg{t}",
                                 tag="gsb", bufs=2)
                for s in range(S):
                    nc.tensor.matmul(
                        seg_ps[0:BDK, 0:K],
                        sel_t[:, BDK * (s * T + t):BDK * (s * T + t + 1)],
                        exp_all[:, K * s:K * (s + 1)],
                        start=(s == 0), stop=(s == S - 1),
                    )
                seg_tiles.append(seg_ps)
            for t in range(T):
                seg_sb = wpool.tile([BDK, K], f32, name="seg_sb", tag="seg_sb")
                nc.vector.tensor_scalar_max(seg_sb[:], seg_tiles[t][0:BDK, 0:K],
                                            1e-30)
                nc.vector.reciprocal(inv_t[:, K * t:K * (t + 1)], seg_sb[:])

            # --- phase D: broadcast-back, B matrix, out = relu(B_aug^T x) ---
            nrelu = 0
            for s in range(S):
                xs = x_tiles[s]
                invb_ps = pp.tile([BDK, K], f32, name="invb", tag="tiny",
                                  bufs=2)
                for t in range(T):
                    nc.tensor.matmul(
                        invb_ps[:],
                        selt_t[:, BDK * (s * T + t):BDK * (s * T + t + 1)],
                        inv_t[:, K * t:K * (t + 1)],
                        start=(t == 0), stop=(t == T - 1),
                    )
                attn_sb = wpool.tile([BDK, K], bf16, name="attn_sb",
                                     tag="attn_c", bufs=2)
                nc.vector.tensor_mul(attn_sb[:], exp_all[:, K * s:K * (s + 1)],
                                     invb_ps[:])
                attn_bd = wpool.tile([BDK, BDK], bf16, name="attn_bd",
                                     tag="attn", bufs=2)
                for j in range(BD):
                    jsl = slice(K * j, K * (j + 1))
                    eng = nc.vector if j % 2 == 0 else nc.gpsimd
                    eng.tensor_mul(attn_bd[:, jsl], attn_sb[:],
                                   mkb_t[:, jsl])
                b_ps = pp.tile([BDK + 1, BDK], f32, name="b_ps", tag="gsb",
                               bufs=2)
                nc.tensor.matmul(b_ps[:], wv_t[:], attn_bd[:], start=True,
                                 stop=True)
                b_sb = wpool.tile([BDK + 1, BDK], bf16, name="b_sb", tag="B",
                                  bufs=2)
                nc.vector.tensor_add(b_sb[:], b_ps[:], ia_t[:])

                osb = opool.tile([BDK, HW], bf16, name="osb", tag="osb",
                                 bufs=2)
                for oc in range(HW // O_CH):
                    sl = slice(O_CH * oc, O_CH * (oc + 1))
                    o_ps = pp.tile([BDK, O_CH], f32, name="o_ps", tag="big",
                                   bufs=4)
                    nc.tensor.matmul(o_ps[:], b_sb[:], xs[:, sl], start=True,
                                     stop=True)
                    if nrelu % 2 == 0:
                        nc.scalar.activation(osb[:, sl], o_ps[:], Relu)
                    else:
                        nc.vector.tensor_scalar_max(osb[:, sl], o_ps[:], 0.0)
                    nrelu += 1
                (nc.sync, nc.gpsimd, nc.scalar)[s % 3].dma_start(
                    y_d.ap()[BDK * s:BDK * (s + 1), :], osb[:]
                )

    nc.compile()
    return nc


def _get_compiled(P_pad: int, T: int, have_bias: bool):
    key = (P_pad, T, have_bias)
    if key not in _cache:
        _cache[key] = _build(P_pad, T, have_bias)
    return _cache[key]


def _bd7(m: np.ndarray) -> np.ndarray:
    out = np.zeros((BDK, BDK), dtype=np.float32)
    for j in range(BD):
        out[K * j:K * (j + 1), K * j:K * (j + 1)] = m
    return out


def _plan(ids: np.ndarray):
    """Split persons into N_CORES contiguous chunks at imgid boundaries."""
    change = np.flatnonzero(np.diff(ids)) + 1
    allb = np.concatenate([[0], change, [P_TOTAL]]).astype(np.int64)
    bounds = [0]
    for ci in range(1, N_CORES):
        target = P_TOTAL * ci / N_CORES
        cand = allb[allb > bounds[-1]]
        if len(cand) == 0:
            bounds.append(bounds[-1])
        else:
            bounds.append(int(cand[np.argmin(np.abs(cand - target))]))
    bounds.append(P_TOTAL)
    sizes = np.diff(bounds)
    P_max = int(sizes.max())
    P_pad = max(BD, BD * math.ceil(P_max / BD))
    ng_max = 1
    for ci in range(N_CORES):
        a, b = bounds[ci], bounds[ci + 1]
        ng_max = max(ng_max, len(np.unique(ids[a:b])) + 1)
    T = math.ceil(ng_max / BD)
    return bounds, P_pad, T


def _prepare(inputs: dict):
    import ml_dtypes
    nbf16 = ml_dtypes.bfloat16

    x = np.asarray(inputs["kpt_feat"], dtype=np.float32).reshape(
        P_TOTAL, K, HW)
    ids = np.asarray(inputs["imgid"]).astype(np.int64)
    Wq = np.asarray(inputs["Wq"], np.float32)
    Wk = np.asarray(inputs["Wk"], np.float32)
    Wv = np.asarray(inputs["Wv"], np.float32)
    bq = np.asarray(inputs["bq"], np.float32)
    bk = np.asarray(inputs["bk"], np.float32)
    bv = np.asarray(inputs["bv"], np.float32)

    bounds, P_pad, T = _plan(ids)
    S = P_pad // BD

    wq_col = np.zeros((BDK, K), np.float32)
    for j in range(BD):
        wq_col[K * j:K * (j + 1), :] = Wq.T / NORM
    wkt_bd = _bd7(Wk.T.astype(np.float32))
    wv_aug = np.zeros((BDK, BDK + 1), np.float32)
    wv_aug[:, :BDK] = _bd7(Wv)
    for j in range(BD):
        wv_aug[K * j:K * (j + 1), BDK] = bv
    wv_aug = wv_aug.astype(nbf16)
    id119 = np.eye(BDK, dtype=np.float32).astype(nbf16)
    iaug = np.zeros((BDK + 1, BDK), np.float32)
    iaug[:BDK, :BDK] = np.eye(BDK, dtype=np.float32)
    bdmask = _bd7(np.ones((K, K), np.float32))

    have_bias = bool(np.any(bq) or np.any(bk))
    if have_bias:
        xsum = x.sum(axis=2)
        qx = xsum @ Wq.T
        kx = xsum @ Wk.T
        corr_all = (bk[None, :, None] * qx[:, None, :]
                    + bq[None, None, :] * kx[:, :, None]
                    + HW * (bq[None, None, :] * bk[None, :, None])) / NORM
        corr_all = corr_all.astype(np.float32)  # [P, m, i]
    else:
        corr_all = None

    xb = x.astype(nbf16)

    in_maps = []
    for ci in range(N_CORES):
        a, b = bounds[ci], bounds[ci + 1]
        pc = b - a
        xs = np.zeros((S * (BDK + 1), HW), dtype=nbf16)
        xs[BDK::BDK + 1] = 1.0  # per-stack all-ones row (residual/bias fold)
        for s in range(S):
            lo, hi = BD * s, min(BD * (s + 1), pc)
            if hi > lo:
                xs[(BDK + 1) * s:(BDK + 1) * s + (hi - lo) * K] = \
                    xb[a + lo:a + hi].reshape((hi - lo) * K, HW)
        # group slots: dummy slot (last) for padding persons
        slots = np.full((P_pad,), 7 * T - 1, np.int64)
        if pc:
            _, lg = np.unique(ids[a:b], return_inverse=True)
            slots[:pc] = lg
        sel = np.zeros((S, T, BDK, BDK), np.float32)
        eye = np.eye(K, dtype=np.float32)
        for s in range(S):
            for j in range(BD):
                g = slots[BD * s + j]
                t, lgi = divmod(g, BD)
                sel[s, t, K * j:K * (j + 1), K * lgi:K * (lgi + 1)] = eye
        # pack as [119, (s*T+t)*119 + c] for a single 2D DMA
        sel_pack = sel.reshape(S * T, BDK, BDK).transpose(1, 0, 2).reshape(
            BDK, S * T * BDK)
        selt_pack = sel.reshape(S * T, BDK, BDK).transpose(2, 0, 1).reshape(
            BDK, S * T * BDK)
        m = {
            "x": xs,
            "wq_col": wq_col,
            "wkt_bd": wkt_bd,
            "wv_aug": wv_aug,
            "id119": id119,
            "iaug": iaug,
            "bdmask": bdmask,
            "bdmaskb": bdmask.astype(nbf16),
            "sel": np.ascontiguousarray(sel_pack),
            "selT": np.ascontiguousarray(selt_pack),
        }
        if have_bias:
            corr_col = np.zeros((BDK, K * S), np.float32)
            if pc:
                cpad = np.zeros((P_pad, K, K), np.float32)
                cpad[:pc] = corr_all[a:b]
                for s in range(S):
                    for j in range(BD):
                        corr_col[K * j:K * (j + 1), K * s:K * (s + 1)] = \
                            cpad[BD * s + j]
            m["corr_col"] = corr_col
        in_maps.append(m)
    return in_maps, bounds, P_pad, T, have_bias


def _gather(results, bounds, P_pad):
    out = np.empty((P_TOTAL, K, 64, 64), dtype=np.float32)
    for ci in range(N_CORES):
        a, b = bounds[ci], bounds[ci + 1]
        pc = b - a
        if pc:
            y = np.asarray(results[ci]["y"][:pc * K], dtype=np.float32)
            out[a:b] = y.reshape(pc, K, 64, 64)
    return out


def _run(inputs: dict, trace: bool = False):
    _ensure_path()
    from concourse.bass_utils import run_bass_kernel_spmd

    in_maps, bounds, P_pad, T, have_bias = _prepare(inputs)
    nc = _get_compiled(P_pad, T, have_bias)
    res = run_bass_kernel_spmd(nc, in_maps, list(range(N_CORES)), trace=trace)
    return _gather(res.results, bounds, P_pad), res


def kernel(**inputs) -> np.ndarray:
    out, _ = _run(inputs, trace=False)
    return out


# revision 28
# speedup vs baseline: 2.3237x; 1.0611x over previous
"""Trainium2 Bass kernel for nn_JointRelationModule (self-contained).

Math (per person p; softmax is segment-softmax over persons within an imgid
group, elementwise over the (K,K) score entries):
    q = Wq x + bq ; k = Wk x + bk ; v = Wv x + bv      (1x1 conv over K=17)
    S_p = q_p k_p^T / 64
    attn = segment-softmax over persons
    out = relu(attn_p @ v_p + x_p)

Device formulation (heavy ops bf16 on the PE, block-column layouts):
  - Stack BD=7 persons as [119, hw]. Per stack: G = x x^T via PE transpose +
    accumulating matmuls (bf16, f32 PSUM).
  - scores^T in block-column layout [119, 17] via a masked-Gram matmul chain
    (block-diag mask kills cross-person terms), so no gather/scatter DMAs.
  - Segment softmax via per-stack selector matmuls into group-slot tiles,
    reciprocal, selector-transpose broadcast back; all partition-aligned.
  - Output: B = blockdiag((attn Wv)^T) + I with an av row appended; the
    residual and v-bias ride along x_aug (all-ones row), so each output chunk
    is one matmul + one relu. Stored bf16, host upcasts.

Data movement: x and y live in a partition-major layout [120, S*hw] so a
multi-stack tile is one DMA with 16KB-contiguous per-partition descriptors
(per-queue DMA throughput here is descriptor-rate-limited); every load/store
is split across the three DMA-capable queues (sync/gpsimd/scalar) by
partition range. Loads are emitted just-in-time with the compute emission.

Sharding: data-parallel over persons at imgid group boundaries (8 cores),
weights replicated. Host casts x to bf16 (halves load bytes); output comes
back bf16 (halves store bytes). Tolerance 2e-2; measured error ~5e-3.
"""

import math
import sys

import numpy as np

K = 17
HW = 4096  # 64*64
P_TOTAL = 512
N_CORES = 8
NORM = 64.0
BD = 7          # persons per stack
BDK = BD * K    # 119
O_CH = 512      # output chunk cols (one PSUM bank of f32)

_cache: dict = {}


def _ensure_path():
    try:
        import concourse.bass  # noqa: F401
    except ImportError:
        for p in ("/opt/trn_rl_repo", "/root/.axon_site/_ro/trn_rl_repo"):
            if p not in sys.path:
                sys.path.insert(0, p)
        import concourse.bass  # noqa: F401


def _build(P_pad: int, T: int, have_bias: bool):
    """Builds + compiles the per-core SPMD Bass program."""
    _ensure_path()
    import concourse.bacc as bacc
    import concourse.mybir as mybir
    import concourse.tile as tile

    f32 = mybir.dt.float32
    bf16 = mybir.dt.bfloat16
    Exp = mybir.ActivationFunctionType.Exp
    Relu = mybir.ActivationFunctionType.Relu

    S = P_pad // BD
    assert P_pad % BD == 0
    # x tile widths in stacks: first two single (fast PE start), then pairs
    widths = [1, 1] + [2] * ((S - 2 + 1) // 2)
    if sum(widths) > S:
        widths[-1] = 1
    assert sum(widths) == S
    starts = np.cumsum([0] + widths).tolist()

    nc = bacc.Bacc(
        "TRN2",
        target_bir_lowering=False,
        debug=False,
        enable_asserts=False,
        num_devices=N_CORES,
    )

    x_d = nc.dram_tensor("x", [BDK + 1, S * HW], bf16, kind="ExternalInput")
    wq_d = nc.dram_tensor("wq_col", [BDK, K], f32, kind="ExternalInput")
    wk_d = nc.dram_tensor("wkt_bd", [BDK, BDK], f32, kind="ExternalInput")
    wv_d = nc.dram_tensor("wv_aug", [BDK, BDK + 1], bf16, kind="ExternalInput")
    id_d = nc.dram_tensor("id119", [BDK, BDK], bf16, kind="ExternalInput")
    ia_d = nc.dram_tensor("iaug", [BDK + 1, BDK], f32, kind="ExternalInput")
    mk_d = nc.dram_tensor("bdmask", [BDK, BDK], f32, kind="ExternalInput")
    mkb_d = nc.dram_tensor("bdmaskb", [BDK, BDK], bf16, kind="ExternalInput")
    sel_d = nc.dram_tensor("sel", [BDK, S * T * BDK], bf16,
                           kind="ExternalInput")
    selt_d = nc.dram_tensor("selT", [BDK, S * T * BDK], bf16,
                            kind="ExternalInput")
    if have_bias:
        corr_d = nc.dram_tensor("corr_col", [BDK, K * S], f32,
                                kind="ExternalInput")
    y_d = nc.dram_tensor("y", [BDK, S * HW], bf16, kind="ExternalOutput")

    with tile.TileContext(nc) as tc:
        with (
            nc.allow_low_precision(reason="bf16 softmax ok at 2e-2 tol"),
            tc.tile_pool(name="xpool", bufs=1) as xpool,
            tc.tile_pool(name="cpool", bufs=1) as cpool,
            tc.tile_pool(name="wpool", bufs=2) as wpool,
            tc.tile_pool(name="opool", bufs=2) as opool,
            tc.tile_pool(name="pp", bufs=2, space="PSUM") as pp,
        ):
            queues = (nc.sync, nc.gpsimd, nc.scalar)
            PSPLIT = (slice(0, 40), slice(40, 80), slice(80, BDK + 1))
            PSPLIT_Y = (slice(0, 40), slice(40, 80), slice(80, BDK))

            # --- tiny phase-A constants first (id_t gates every transpose) ---
            id_t = cpool.tile([BDK, BDK], bf16, name="id_t", tag="id")
            mk_t = cpool.tile([BDK, BDK], f32, name="mk_t", tag="mk")
            wq_t = cpool.tile([BDK, K], f32, name="wq_t", tag="wq")
            wk_t = cpool.tile([BDK, BDK], f32, name="wk_t", tag="wk")
            nc.sync.dma_start(id_t[:], id_d.ap())
            nc.gpsimd.dma_start(mk_t[:], mk_d.ap())
            nc.sync.dma_start(wq_t[:], wq_d.ap())
            nc.gpsimd.dma_start(wk_t[:], wk_d.ap())

            x_tiles = []  # per x-tile (1 or 2 stacks wide)

            def load_xtile(k):
                w = widths[k]
                xt_ = xpool.tile([BDK + 1, w * HW], bf16, name=f"xp{k}",
                                 tag=f"xp{k}")
                csl = slice(starts[k] * HW, (starts[k] + w) * HW)
                for qi, psl in enumerate(PSPLIT):
                    queues[qi].dma_start(xt_[psl, :], x_d.ap()[psl, csl])
                x_tiles.append(xt_)

            def xs_ap(s):
                k = next(i for i in range(len(widths))
                         if starts[i] <= s < starts[i + 1])
                off = (s - starts[k]) * HW
                return x_tiles[k], off

            load_xtile(0)
            load_xtile(1)
            load_xtile(2)

            # bulkier constants (needed from phase C on)
            wv_t = cpool.tile([BDK, BDK + 1], bf16, name="wv_t", tag="wv")
            ia_t = cpool.tile([BDK + 1, BDK], f32, name="ia_t", tag="ia")
            sel_t = cpool.tile([BDK, S * T * BDK], bf16, name="sel_t",
                               tag="sel")
            selt_t = cpool.tile([BDK, S * T * BDK], bf16, name="selt_t",
                                tag="selt")
            mkb_t = cpool.tile([BDK, BDK], bf16, name="mkb_t", tag="mkb")
            nc.sync.dma_start(wv_t[:], wv_d.ap())
            nc.gpsimd.dma_start(ia_t[:], ia_d.ap())
            nc.scalar.dma_start(mkb_t[:], mkb_d.ap())
            nc.sync.dma_start(sel_t[:], sel_d.ap())
            nc.gpsimd.dma_start(selt_t[:], selt_d.ap())
            if have_bias:
                corr_t = cpool.tile([BDK, K * S], f32, name="corr_t",
                                    tag="corr")
                nc.scalar.dma_start(corr_t[:], corr_d.ap())

            exp_all = cpool.tile([BDK, K * S], bf16, name="exp_all", tag="exp")
            inv_t = cpool.tile([BDK, K * T], bf16, name="inv_t", tag="inv")

            # --- phase A: transpose -> gram -> scores^T -> exp, skewed ---
            # PSUM tags (8 banks): big=tp/o_ps x4, gsb=g/seg/b x2, tiny x2
            G_CH = 1024          # x cols per transpose group
            n_grp = HW // G_CH   # 4 groups per stack
            TC = BDK + 1         # 120: chunk col stride (4B-aligned in PSUM)
            state = {"ncopy": 0, "loaded": 3}
            g_tiles = {}

            def emit_transposes(s, gi):
                xt_, off = xs_ap(s)
                tp = pp.tile([128, 8 * TC], bf16, name="tp", tag="big",
                             bufs=4)
                for c8 in range(8):
                    col = off + G_CH * gi + 128 * c8
                    nc.tensor.transpose(
                        tp[:, TC * c8:TC * c8 + BDK],
                        xt_[0:BDK, col:col + 128], id_t[:],
                    )
                xt = wpool.tile([128, 8 * TC], bf16, name="xt", tag="xt",
                                bufs=4)
                if state["ncopy"] % 2 == 0:
                    nc.vector.tensor_copy(xt[:], tp[:])
                else:
                    nc.scalar.copy(xt[:], tp[:])
                state["ncopy"] += 1
                return xt

            def emit_gram(s, gi, xt):
                if s not in g_tiles:
                    g_tiles[s] = pp.tile([BDK + 1, BDK], f32, name=f"g{s}",
                                         tag="gsb", bufs=2)
                g_ps = g_tiles[s]
                for c8 in range(8):
                    nc.tensor.matmul(
                        g_ps[0:BDK, :], xt[:, TC * c8:TC * c8 + BDK],
                        xt[:, TC * c8:TC * c8 + BDK],
                        start=(gi == 0 and c8 == 0),
                        stop=(gi == n_grp - 1 and c8 == 7),
                    )
                if gi == n_grp - 1:
                    emit_tiny_chain(s)

            def emit_tiny_chain(s):
                g_sb = wpool.tile([BDK, BDK], f32, name="g_sb", tag="g_sb",
                                  bufs=2)
                nc.vector.tensor_mul(g_sb[:], g_tiles[s][0:BDK, :], mk_t[:])
                m1_ps = pp.tile([BDK, K], f32, name="m1", tag="tiny", bufs=2)
                nc.tensor.matmul(m1_ps[:], g_sb[:], wq_t[:], start=True,
                                 stop=True)
                m1_sb = wpool.tile([BDK, K], f32, name="m1_sb", tag="m1_sb",
                                   bufs=2)
                nc.scalar.copy(m1_sb[:], m1_ps[:])
                st_ps = pp.tile([BDK, K], f32, name="st", tag="tiny", bufs=2)
                nc.tensor.matmul(st_ps[:], wk_t[:], m1_sb[:], start=True,
                                 stop=True)
                esl = slice(K * s, K * (s + 1))
                if have_bias:
                    eb_sb = wpool.tile([BDK, K], f32, name="eb_sb", tag="eb")
                    nc.vector.tensor_add(eb_sb[:], st_ps[:], corr_t[:, esl])
                    nc.scalar.activation(exp_all[:, esl], eb_sb[:], Exp)
                else:
                    nc.scalar.activation(exp_all[:, esl], st_ps[:], Exp)

            pend = []
            for s in range(S):
                # just-in-time prefetch: issue the load 2 tiles ahead
                k = next(i for i in range(len(widths))
                         if starts[i] <= s < starts[i + 1])
                while state["loaded"] < min(k + 3, len(widths)):
                    load_xtile(state["loaded"])
                    state["loaded"] += 1
                for gi in range(n_grp):
                    pend.append((s, gi, emit_transposes(s, gi)))
                    if len(pend) > 2:
                        ps, pgi, xt = pend.pop(0)
                        emit_gram(ps, pgi, xt)
            for ps, pgi, xt in pend:
                emit_gram(ps, pgi, xt)

            # --- phase C: segment sums -> reciprocal ---
            seg_tiles = []
            for t in range(T):
                seg_ps = pp.tile([BDK + 1, BDK], f32, name=f"seg{t}",
                                 tag="gsb", bufs=2)
                for s in range(S):
                    nc.tensor.matmul(
                        seg_ps[0:BDK, 0:K],
                        sel_t[:, BDK * (s * T + t):BDK * (s * T + t + 1)],
                        exp_all[:, K * s:K * (s + 1)],
                        start=(s == 0), stop=(s == S - 1),
                    )
                seg_tiles.append(seg_ps)
            for t in range(T):
                seg_sb = wpool.tile([BDK, K], f32, name="seg_sb",
                                    tag="seg_sb")
                nc.vector.tensor_scalar_max(seg_sb[:],
                                            seg_tiles[t][0:BDK, 0:K], 1e-30)
                nc.vector.reciprocal(inv_t[:, K * t:K * (t + 1)], seg_sb[:])

            # --- phase D: pipelined per stack ---
            nrelu = 0
            attn_tiles = {}

            def emit_attn_chain(s):
                invb_ps = pp.tile([BDK, K], f32, name="invb", tag="tiny",
                                  bufs=2)
                for t in range(T):
                    nc.tensor.matmul(
                        invb_ps[:],
                        selt_t[:, BDK * (s * T + t):BDK * (s * T + t + 1)],
                        inv_t[:, K * t:K * (t + 1)],
                        start=(t == 0), stop=(t == T - 1),
                    )
                attn_sb = wpool.tile([BDK, K], bf16, name="attn_sb",
                                     tag="attn_c", bufs=2)
                nc.vector.tensor_mul(attn_sb[:], exp_all[:, K * s:K * (s + 1)],
                                     invb_ps[:])
                attn_bd = wpool.tile([BDK, BDK], bf16, name="attn_bd",
                                     tag="attn", bufs=2)
                for j in range(BD):
                    jsl = slice(K * j, K * (j + 1))
                    eng = nc.gpsimd if j % 2 == 0 else nc.vector
                    eng.tensor_mul(attn_bd[:, jsl], attn_sb[:], mkb_t[:, jsl])
                attn_tiles[s] = attn_bd

            emit_attn_chain(0)
            osb = None
            for s in range(S):
                b_ps = pp.tile([BDK + 1, BDK], f32, name="b_ps", tag="gsb",
                               bufs=2)
                nc.tensor.matmul(b_ps[:], wv_t[:], attn_tiles.pop(s)[:],
                                 start=True, stop=True)
                b_sb = wpool.tile([BDK + 1, BDK], bf16, name="b_sb", tag="B",
                                  bufs=2)
                nc.vector.tensor_add(b_sb[:], b_ps[:], ia_t[:])
                if s + 1 < S:
                    emit_attn_chain(s + 1)  # overlaps this stack's matmuls

                k = next(i for i in range(len(widths))
                         if starts[i] <= s < starts[i + 1])
                first_in_tile = (s == starts[k])
                if first_in_tile:
                    osb = opool.tile([BDK, widths[k] * HW], bf16, name="osb",
                                     tag="osb", bufs=2)
                xt_, off = xs_ap(s)
                for oc in range(HW // O_CH):
                    sl = slice(off + O_CH * oc, off + O_CH * (oc + 1))
                    o_ps = pp.tile([BDK, O_CH], f32, name="o_ps", tag="big",
                                   bufs=4)
                    nc.tensor.matmul(o_ps[:], b_sb[:], xt_[:, sl],
                                     start=True, stop=True)
                    if nrelu % 2 == 0:
                        nc.scalar.activation(osb[:, sl], o_ps[:], Relu)
                    else:
                        nc.vector.tensor_scalar_max(osb[:, sl], o_ps[:], 0.0)
                    nrelu += 1
                if s + 1 == starts[k] + widths[k]:  # tile complete -> store
                    csl = slice(starts[k] * HW, (starts[k] + widths[k]) * HW)
                    for qi, psl in enumerate(PSPLIT_Y):
                        queues[qi].dma_start(y_d.ap()[psl, csl], osb[psl, :])

    nc.compile()
    return nc


def _get_compiled(P_pad: int, T: int, have_bias: bool):
    key = (P_pad, T, have_bias)
    if key not in _cache:
        _cache[key] = _build(P_pad, T, have_bias)
    return _cache[key]


def _bd7(m: np.ndarray) -> np.ndarray:
    out = np.zeros((BDK, BDK), dtype=np.float32)
    for j in range(BD):
        out[K * j:K * (j + 1), K * j:K * (j + 1)] = m
    return out


def _plan(ids: np.ndarray):
    """Split persons into N_CORES contiguous chunks at imgid boundaries."""
    change = np.flatnonzero(np.diff(ids)) + 1
    allb = np.concatenate([[0], change, [P_TOTAL]]).astype(np.int64)
    bounds = [0]
    for ci in range(1, N_CORES):
        target = P_TOTAL * ci / N_CORES
        cand = allb[allb > bounds[-1]]
        if len(cand) == 0:
            bounds.append(bounds[-1])
        else:
            bounds.append(int(cand[np.argmin(np.abs(cand - target))]))
    bounds.append(P_TOTAL)
    sizes = np.diff(bounds)
    P_max = int(sizes.max())
    P_pad = max(BD, BD * math.ceil(P_max / BD))
    ng_max = 1
    for ci in range(N_CORES):
        a, b = bounds[ci], bounds[ci + 1]
        ng_max = max(ng_max, len(np.unique(ids[a:b])) + 1)
    T = math.ceil(ng_max / BD)
    return bounds, P_pad, T


def _prepare(inputs: dict):
    import ml_dtypes
    nbf16 = ml_dtypes.bfloat16

    x = np.asarray(inputs["kpt_feat"], dtype=np.float32).reshape(
        P_TOTAL, K, HW)
    ids = np.asarray(inputs["imgid"]).astype(np.int64)
    Wq = np.asarray(inputs["Wq"], np.float32)
    Wk = np.asarray(inputs["Wk"], np.float32)
    Wv = np.asarray(inputs["Wv"], np.float32)
    bq = np.asarray(inputs["bq"], np.float32)
    bk = np.asarray(inputs["bk"], np.float32)
    bv = np.asarray(inputs["bv"], np.float32)

    bounds, P_pad, T = _plan(ids)
    S = P_pad // BD

    wq_col = np.zeros((BDK, K), np.float32)
    for j in range(BD):
        wq_col[K * j:K * (j + 1), :] = Wq.T / NORM
    wkt_bd = _bd7(Wk.T.astype(np.float32))
    wv_aug = np.zeros((BDK, BDK + 1), np.float32)
    wv_aug[:, :BDK] = _bd7(Wv)
    for j in range(BD):
        wv_aug[K * j:K * (j + 1), BDK] = bv
    wv_aug = wv_aug.astype(nbf16)
    id119 = np.eye(BDK, dtype=np.float32).astype(nbf16)
    iaug = np.zeros((BDK + 1, BDK), np.float32)
    iaug[:BDK, :BDK] = np.eye(BDK, dtype=np.float32)
    bdmask = _bd7(np.ones((K, K), np.float32))

    have_bias = bool(np.any(bq) or np.any(bk))
    if have_bias:
        xsum = x.sum(axis=2)
        qx = xsum @ Wq.T
        kx = xsum @ Wk.T
        corr_all = (bk[None, :, None] * qx[:, None, :]
                    + bq[None, None, :] * kx[:, :, None]
                    + HW * (bq[None, None, :] * bk[None, :, None])) / NORM
        corr_all = corr_all.astype(np.float32)  # [P, m, i]
    else:
        corr_all = None

    xb = x.astype(nbf16)

    in_maps = []
    for ci in range(N_CORES):
        a, b = bounds[ci], bounds[ci + 1]
        pc = b - a
        # partition-major x: [120, S*HW]; row 119 = ones (residual fold)
        rows = np.zeros((P_pad * K, HW), dtype=nbf16)
        if pc:
            rows[:pc * K] = xb[a:b].reshape(pc * K, HW)
        arr3 = np.zeros((S, BDK + 1, HW), dtype=nbf16)
        arr3[:, :BDK] = rows.reshape(S, BDK, HW)
        arr3[:, BDK] = 1.0
        xs = np.ascontiguousarray(
            arr3.transpose(1, 0, 2).reshape(BDK + 1, S * HW))
        # group slots: dummy slot (last) for padding persons
        slots = np.full((P_pad,), 7 * T - 1, np.int64)
        if pc:
            _, lg = np.unique(ids[a:b], return_inverse=True)
            slots[:pc] = lg
        sel = np.zeros((S, T, BDK, BDK), np.float32)
        eye = np.eye(K, dtype=np.float32)
        for s in range(S):
            for j in range(BD):
                g = slots[BD * s + j]
                t, lgi = divmod(g, BD)
                sel[s, t, K * j:K * (j + 1), K * lgi:K * (lgi + 1)] = eye
        sel_pack = sel.reshape(S * T, BDK, BDK).transpose(1, 0, 2).reshape(
            BDK, S * T * BDK)
        selt_pack = sel.reshape(S * T, BDK, BDK).transpose(2, 0, 1).reshape(
            BDK, S * T * BDK)
        m = {
            "x": xs,
            "wq_col": wq_col,
            "wkt_bd": wkt_bd,
            "wv_aug": wv_aug,
            "id119": id119,
            "iaug": iaug,
            "bdmask": bdmask,
            "bdmaskb": bdmask.astype(nbf16),
            "sel": np.ascontiguousarray(sel_pack).astype(nbf16),
            "selT": np.ascontiguousarray(selt_pack).astype(nbf16),
        }
        if have_bias:
            corr_col = np.zeros((BDK, K * S), np.float32)
            if pc:
                cpad = np.zeros((P_pad, K, K), np.float32)
                cpad[:pc] = corr_all[a:b]
                for s in range(S):
                    for j in range(BD):
                        corr_col[K * j:K * (j + 1), K * s:K * (s + 1)] = \
                            cpad[BD * s + j]
            m["corr_col"] = corr_col
        in_maps.append(m)
    return in_maps, bounds, P_pad, T, have_bias


def _gather(results, bounds, P_pad):
    S = P_pad // BD
    out = np.empty((P_TOTAL, K, 64, 64), dtype=np.float32)
    for ci in range(N_CORES):
        a, b = bounds[ci], bounds[ci + 1]
        pc = b - a
        if pc:
            y = np.asarray(results[ci]["y"], dtype=np.float32)  # [119, S*HW]
            y = y.reshape(BDK, S, HW).transpose(1, 0, 2).reshape(
                P_pad, K, 64, 64)
            out[a:b] = y[:pc]
    return out


def _run(inputs: dict, trace: bool = False):
    _ensure_path()
    from concourse.bass_utils import run_bass_kernel_spmd

    in_maps, bounds, P_pad, T, have_bias = _prepare(inputs)
    nc = _get_compiled(P_pad, T, have_bias)
    res = run_bass_kernel_spmd(nc, in_maps, list(range(N_CORES)), trace=trace)
    return _gather(res.results, bounds, P_pad), res


def kernel(**inputs) -> np.ndarray:
    out, _ = _run(inputs, trace=False)
    return out


# revision 40
# speedup vs baseline: 2.5415x; 1.0937x over previous
"""Trainium2 Bass kernel for nn_JointRelationModule (self-contained).

Math (per person p; softmax is segment-softmax over persons within an imgid
group, elementwise over the (K,K) score entries):
    q = Wq x + bq ; k = Wk x + bk ; v = Wv x + bv      (1x1 conv over K=17)
    S_p = q_p k_p^T / 64
    attn = segment-softmax over persons
    out = relu(attn_p @ v_p + x_p)

Device formulation (heavy ops bf16 on the PE, block-column layouts):
  - Stack BD=7 persons as [119, hw]. Per stack: G = x x^T via PE transpose +
    accumulating matmuls (bf16, f32 PSUM).
  - scores^T in block-column layout [119, 17] via a masked-Gram matmul chain
    (block-diag mask kills cross-person terms), so no gather/scatter DMAs.
  - Segment softmax via per-stack selector matmuls into group-slot tiles,
    reciprocal, selector-transpose broadcast back; all partition-aligned.
  - Output: B = blockdiag((attn Wv)^T) + I with an av row appended; the
    residual and v-bias ride along x_aug (all-ones row), so each output chunk
    is one matmul + one relu. Stored bf16, host upcasts.

Data movement: x and y live in a partition-major layout [120, S*hw] so a
multi-stack tile is one DMA with 16KB-contiguous per-partition descriptors
(per-queue DMA throughput here is descriptor-rate-limited); every load/store
is split across the three DMA-capable queues (sync/gpsimd/scalar) by
partition range. Loads are emitted just-in-time with the compute emission.

Sharding: data-parallel over persons at imgid group boundaries (8 cores),
weights replicated. Host casts x to bf16 (halves load bytes); output comes
back bf16 (halves store bytes). Tolerance 2e-2; measured error ~5e-3.
"""

import math
import sys

import numpy as np

K = 17
HW = 4096  # 64*64
P_TOTAL = 512
N_CORES = 8
NORM = 64.0
BD = 7          # persons per stack
BDK = BD * K    # 119
O_CH = 512      # output chunk cols (one PSUM bank of f32)

_cache: dict = {}


def _ensure_path():
    try:
        import concourse.bass  # noqa: F401
    except ImportError:
        for p in ("/opt/trn_rl_repo", "/root/.axon_site/_ro/trn_rl_repo"):
            if p not in sys.path:
                sys.path.insert(0, p)
        import concourse.bass  # noqa: F401


def _build(P_pad: int, T: int, have_bias: bool, used: tuple):
    """Builds + compiles the per-core SPMD Bass program."""
    _ensure_path()
    import concourse.bacc as bacc
    import concourse.mybir as mybir
    import concourse.tile as tile

    f32 = mybir.dt.float32
    bf16 = mybir.dt.bfloat16
    Exp = mybir.ActivationFunctionType.Exp
    Relu = mybir.ActivationFunctionType.Relu

    S = P_pad // BD
    assert P_pad % BD == 0
    U = len(used)

    nc = bacc.Bacc(
        "TRN2",
        target_bir_lowering=False,
        debug=False,
        enable_asserts=False,
        num_devices=N_CORES,
    )

    x_d = nc.dram_tensor("x", [BDK + 1, S * HW], bf16, kind="ExternalInput")
    wq_d = nc.dram_tensor("wq_col", [BDK, K], f32, kind="ExternalInput")
    wk_d = nc.dram_tensor("wkt_bd", [BDK, BDK], f32, kind="ExternalInput")
    wv_d = nc.dram_tensor("wv_aug", [BDK, BDK + 1], bf16, kind="ExternalInput")
    id_d = nc.dram_tensor("id119", [BDK, BDK], bf16, kind="ExternalInput")
    ia_d = nc.dram_tensor("iaug", [BDK + 1, BDK], f32, kind="ExternalInput")
    mk_d = nc.dram_tensor("bdmask", [BDK, BDK], f32, kind="ExternalInput")
    mkb_d = nc.dram_tensor("bdmaskb", [BDK, BDK], bf16, kind="ExternalInput")
    sel_d = nc.dram_tensor("sel", [BDK, U * BDK], bf16, kind="ExternalInput")
    selt_d = nc.dram_tensor("selT", [BDK, U * BDK], bf16,
                            kind="ExternalInput")
    if have_bias:
        corr_d = nc.dram_tensor("corr_col", [BDK, K * S], f32,
                                kind="ExternalInput")
    y_d = nc.dram_tensor("y", [BDK, S * HW], bf16, kind="ExternalOutput")

    with tile.TileContext(nc) as tc:
        with (
            nc.allow_low_precision(reason="bf16 softmax ok at 2e-2 tol"),
            tc.tile_pool(name="xpool", bufs=1) as xpool,
            tc.tile_pool(name="cpool", bufs=1) as cpool,
            tc.tile_pool(name="wpool", bufs=2) as wpool,
            tc.tile_pool(name="opool", bufs=2) as opool,
            tc.tile_pool(name="pp", bufs=2, space="PSUM") as pp,
        ):
            queues = (nc.sync, nc.gpsimd, nc.scalar)
            PSPLIT = (slice(0, 40), slice(40, 80), slice(80, BDK + 1))
            PSPLIT_Y = (slice(0, 40), slice(40, 80), slice(80, BDK))

            # --- tiny phase-A constants first (id_t gates every transpose) ---
            id_t = cpool.tile([BDK, BDK], bf16, name="id_t", tag="id")
            mk_t = cpool.tile([BDK, BDK], f32, name="mk_t", tag="mk")
            wq_t = cpool.tile([BDK, K], f32, name="wq_t", tag="wq")
            wk_t = cpool.tile([BDK, BDK], f32, name="wk_t", tag="wk")
            nc.sync.dma_start(id_t[:], id_d.ap())
            nc.gpsimd.dma_start(mk_t[:], mk_d.ap())
            nc.sync.dma_start(wq_t[:], wq_d.ap())
            nc.gpsimd.dma_start(wk_t[:], wk_d.ap())

            x_tiles = []  # per stack

            def load_xtile(s):
                xt_ = xpool.tile([BDK + 1, HW], bf16, name=f"xp{s}",
                                 tag=f"xp{s}")
                csl = slice(s * HW, (s + 1) * HW)
                for qi, psl in enumerate(PSPLIT):
                    queues[qi].dma_start(xt_[psl, :], x_d.ap()[psl, csl])
                x_tiles.append(xt_)

            load_xtile(0)
            if have_bias:
                corr_t = cpool.tile([BDK, K * S], f32, name="corr_t",
                                    tag="corr")
                nc.scalar.dma_start(corr_t[:], corr_d.ap())

            # bulkier constants: tiles declared now, DMAs emitted mid-phase-A
            # (the framework coalesces DMA waits into a cumulative counter, so
            # anything emitted before the first transpose delays it)
            wv_t = cpool.tile([BDK, BDK + 1], bf16, name="wv_t", tag="wv")
            ia_t = cpool.tile([BDK + 1, BDK], f32, name="ia_t", tag="ia")
            sel_t = cpool.tile([BDK, U * BDK], bf16, name="sel_t", tag="sel")
            selt_t = cpool.tile([BDK, U * BDK], bf16, name="selt_t",
                                tag="selt")
            mkb_t = cpool.tile([BDK, BDK], bf16, name="mkb_t", tag="mkb")

            def emit_const_dmas():
                nc.sync.dma_start(wv_t[:], wv_d.ap())
                nc.gpsimd.dma_start(ia_t[:], ia_d.ap())
                nc.scalar.dma_start(mkb_t[:], mkb_d.ap())
                nc.sync.dma_start(sel_t[:], sel_d.ap())
                nc.gpsimd.dma_start(selt_t[:], selt_d.ap())

            exp_all = cpool.tile([BDK, K * S], bf16, name="exp_all", tag="exp")
            inv_t = cpool.tile([BDK, K * T], bf16, name="inv_t", tag="inv")

            # --- phase A: transpose -> gram -> scores^T -> exp, skewed ---
            # PSUM tags (8 banks): big=tp/o_ps x4, gsb=g/seg/b x2, tiny x2
            G_CH = 1024          # x cols per transpose group
            n_grp = HW // G_CH   # 4 groups per stack
            TC = BDK + 1         # 120: chunk col stride (4B-aligned in PSUM)
            state = {"ncopy": 0, "loaded": 1}
            g_tiles = {}

            def emit_transposes(s, gi):
                xt_ = x_tiles[s]
                tp = pp.tile([128, 8 * TC], bf16, name="tp", tag="big",
                             bufs=4)
                for c8 in range(8):
                    col = G_CH * gi + 128 * c8
                    nc.tensor.transpose(
                        tp[:, TC * c8:TC * c8 + BDK],
                        xt_[0:BDK, col:col + 128], id_t[:],
                    )
                xt = wpool.tile([128, 8 * TC], bf16, name="xt", tag="xt",
                                bufs=4)
                if state["ncopy"] % 2 == 0:
                    nc.vector.tensor_copy(xt[:], tp[:])
                else:
                    nc.scalar.copy(xt[:], tp[:])
                state["ncopy"] += 1
                return xt

            def emit_gram(s, gi, xt):
                if s not in g_tiles:
                    g_tiles[s] = pp.tile([BDK + 1, BDK], f32, name=f"g{s}",
                                         tag="gsb", bufs=2)
                g_ps = g_tiles[s]
                for c8 in range(8):
                    nc.tensor.matmul(
                        g_ps[0:BDK, :], xt[:, TC * c8:TC * c8 + BDK],
                        xt[:, TC * c8:TC * c8 + BDK],
                        start=(gi == 0 and c8 == 0),
                        stop=(gi == n_grp - 1 and c8 == 7),
                    )
                if gi == n_grp - 1:
                    emit_tiny_chain(s)

            def emit_tiny_chain(s):
                g_sb = wpool.tile([BDK, BDK], f32, name="g_sb", tag="g_sb",
                                  bufs=2)
                nc.vector.tensor_mul(g_sb[:], g_tiles[s][0:BDK, :], mk_t[:])
                m1_ps = pp.tile([BDK, K], f32, name="m1", tag="tiny", bufs=2)
                nc.tensor.matmul(m1_ps[:], g_sb[:], wq_t[:], start=True,
                                 stop=True)
                m1_sb = wpool.tile([BDK, K], f32, name="m1_sb", tag="m1_sb",
                                   bufs=2)
                nc.scalar.copy(m1_sb[:], m1_ps[:])
                st_ps = pp.tile([BDK, K], f32, name="st", tag="tiny", bufs=2)
                nc.tensor.matmul(st_ps[:], wk_t[:], m1_sb[:], start=True,
                                 stop=True)
                esl = slice(K * s, K * (s + 1))
                if have_bias:
                    eb_sb = wpool.tile([BDK, K], f32, name="eb_sb", tag="eb")
                    nc.vector.tensor_add(eb_sb[:], st_ps[:], corr_t[:, esl])
                    nc.scalar.activation(exp_all[:, esl], eb_sb[:], Exp)
                else:
                    nc.scalar.activation(exp_all[:, esl], st_ps[:], Exp)

            pend = []
            for s in range(S):
                if s == 2:
                    emit_const_dmas()
                for gi in range(n_grp):
                    pend.append((s, gi, emit_transposes(s, gi)))
                    # just-in-time prefetch, interleaved with compute emission
                    while state["loaded"] < min(s + 3, S):
                        load_xtile(state["loaded"])
                        state["loaded"] += 1
                    if len(pend) > 2:
                        ps, pgi, xt = pend.pop(0)
                        emit_gram(ps, pgi, xt)
            for ps, pgi, xt in pend:
                emit_gram(ps, pgi, xt)

            # --- phase C: segment sums -> reciprocal ---
            seg_tiles = []
            for t in range(T):
                idxs = [i for i, (ss, tt) in enumerate(used) if tt == t]
                seg_ps = pp.tile([BDK + 1, BDK], f32, name=f"seg{t}",
                                 tag="gsb", bufs=2)
                for n, i in enumerate(idxs):
                    s = used[i][0]
                    nc.tensor.matmul(
                        seg_ps[0:BDK, 0:K],
                        sel_t[:, BDK * i:BDK * (i + 1)],
                        exp_all[:, K * s:K * (s + 1)],
                        start=(n == 0), stop=(n == len(idxs) - 1),
                    )
                seg_tiles.append(seg_ps)
            for t in range(T):
                seg_sb = wpool.tile([BDK, K], f32, name="seg_sb",
                                    tag="seg_sb")
                nc.vector.tensor_scalar_max(seg_sb[:],
                                            seg_tiles[t][0:BDK, 0:K], 1e-30)
                nc.vector.reciprocal(inv_t[:, K * t:K * (t + 1)], seg_sb[:])

            # --- phase D: pipelined per stack ---
            nrelu = 0
            attn_tiles = {}

            def emit_attn_chain(s):
                idxs = [i for i, (ss, tt) in enumerate(used) if ss == s]
                invb_ps = pp.tile([BDK, K], f32, name="invb", tag="tiny",
                                  bufs=2)
                for n, i in enumerate(idxs):
                    t = used[i][1]
                    nc.tensor.matmul(
                        invb_ps[:],
                        selt_t[:, BDK * i:BDK * (i + 1)],
                        inv_t[:, K * t:K * (t + 1)],
                        start=(n == 0), stop=(n == len(idxs) - 1),
                    )
                attn_sb = wpool.tile([BDK, K], bf16, name="attn_sb",
                                     tag="attn_c", bufs=2)
                nc.vector.tensor_mul(attn_sb[:], exp_all[:, K * s:K * (s + 1)],
                                     invb_ps[:])
                attn_bd = wpool.tile([BDK, BDK], bf16, name="attn_bd",
                                     tag="attn", bufs=2)
                for j in range(BD):
                    jsl = slice(K * j, K * (j + 1))
                    eng = nc.gpsimd if j % 2 == 0 else nc.vector
                    eng.tensor_mul(attn_bd[:, jsl], attn_sb[:], mkb_t[:, jsl])
                attn_tiles[s] = attn_bd

            emit_attn_chain(0)
            osb = None
            for s in range(S):
                b_ps = pp.tile([BDK + 1, BDK], f32, name="b_ps", tag="gsb",
                               bufs=2)
                nc.tensor.matmul(b_ps[:], wv_t[:], attn_tiles.pop(s)[:],
                                 start=True, stop=True)
                b_sb = wpool.tile([BDK + 1, BDK], bf16, name="b_sb", tag="B",
                                  bufs=2)
                nc.vector.tensor_add(b_sb[:], b_ps[:], ia_t[:])
                if s + 1 < S:
                    emit_attn_chain(s + 1)  # overlaps this stack's matmuls

                osb = opool.tile([BDK, HW], bf16, name="osb", tag="osb",
                                 bufs=3)
                xt_ = x_tiles[s]
                for oc in range(HW // O_CH):
                    sl = slice(O_CH * oc, O_CH * (oc + 1))
                    o_ps = pp.tile([BDK, O_CH], f32, name="o_ps", tag="big",
                                   bufs=4)
                    nc.tensor.matmul(o_ps[:], b_sb[:], xt_[:, sl],
                                     start=True, stop=True)
                    if nrelu % 2 == 0:
                        nc.scalar.activation(osb[:, sl], o_ps[:], Relu)
                    else:
                        nc.vector.tensor_scalar_max(osb[:, sl], o_ps[:], 0.0)
                    nrelu += 1
                queues[s % 3].dma_start(
                    y_d.ap()[:, s * HW:(s + 1) * HW], osb[:])

    nc.compile()
    return nc


def _get_compiled(P_pad: int, T: int, have_bias: bool, used: tuple):
    key = (P_pad, T, have_bias, used)
    if key not in _cache:
        _cache[key] = _build(P_pad, T, have_bias, used)
    return _cache[key]


def _bd7(m: np.ndarray) -> np.ndarray:
    out = np.zeros((BDK, BDK), dtype=np.float32)
    for j in range(BD):
        out[K * j:K * (j + 1), K * j:K * (j + 1)] = m
    return out


def _plan(ids: np.ndarray):
    """Split persons into N_CORES contiguous chunks at imgid boundaries."""
    change = np.flatnonzero(np.diff(ids)) + 1
    allb = np.concatenate([[0], change, [P_TOTAL]]).astype(np.int64)
    bounds = [0]
    for ci in range(1, N_CORES):
        target = P_TOTAL * ci / N_CORES
        cand = allb[allb > bounds[-1]]
        if len(cand) == 0:
            bounds.append(bounds[-1])
        else:
            bounds.append(int(cand[np.argmin(np.abs(cand - target))]))
    bounds.append(P_TOTAL)
    sizes = np.diff(bounds)
    P_max = int(sizes.max())
    P_pad = max(BD, BD * math.ceil(P_max / BD))
    ng_max = 1
    for ci in range(N_CORES):
        a, b = bounds[ci], bounds[ci + 1]
        ng_max = max(ng_max, len(np.unique(ids[a:b])) + 1)
    T = math.ceil(ng_max / BD)
    return bounds, P_pad, T


def _prepare(inputs: dict):
    import ml_dtypes
    nbf16 = ml_dtypes.bfloat16

    x = np.asarray(inputs["kpt_feat"], dtype=np.float32).reshape(
        P_TOTAL, K, HW)
    ids = np.asarray(inputs["imgid"]).astype(np.int64)
    Wq = np.asarray(inputs["Wq"], np.float32)
    Wk = np.asarray(inputs["Wk"], np.float32)
    Wv = np.asarray(inputs["Wv"], np.float32)
    bq = np.asarray(inputs["bq"], np.float32)
    bk = np.asarray(inputs["bk"], np.float32)
    bv = np.asarray(inputs["bv"], np.float32)

    bounds, P_pad, T = _plan(ids)
    S = P_pad // BD

    wq_col = np.zeros((BDK, K), np.float32)
    for j in range(BD):
        wq_col[K * j:K * (j + 1), :] = Wq.T / NORM
    wkt_bd = _bd7(Wk.T.astype(np.float32))
    wv_aug = np.zeros((BDK, BDK + 1), np.float32)
    wv_aug[:, :BDK] = _bd7(Wv)
    for j in range(BD):
        wv_aug[K * j:K * (j + 1), BDK] = bv
    wv_aug = wv_aug.astype(nbf16)
    id119 = np.eye(BDK, dtype=np.float32).astype(nbf16)
    iaug = np.zeros((BDK + 1, BDK), np.float32)
    iaug[:BDK, :BDK] = np.eye(BDK, dtype=np.float32)
    bdmask = _bd7(np.ones((K, K), np.float32))

    have_bias = bool(np.any(bq) or np.any(bk))
    if have_bias:
        xsum = x.sum(axis=2)
        qx = xsum @ Wq.T
        kx = xsum @ Wk.T
        corr_all = (bk[None, :, None] * qx[:, None, :]
                    + bq[None, None, :] * kx[:, :, None]
                    + HW * (bq[None, None, :] * bk[None, :, None])) / NORM
        corr_all = corr_all.astype(np.float32)  # [P, m, i]
    else:
        corr_all = None

    xb = x.astype(nbf16)

    # selector tensors per core + union of nonzero (s, t) pairs
    eye = np.eye(K, dtype=np.float32)
    sels = []
    used_set = set()
    for ci in range(N_CORES):
        a, b = bounds[ci], bounds[ci + 1]
        pc = b - a
        slots = np.full((P_pad,), 7 * T - 1, np.int64)
        if pc:
            _, lg = np.unique(ids[a:b], return_inverse=True)
            slots[:pc] = lg
        sel = np.zeros((S, T, BDK, BDK), np.float32)
        for s in range(S):
            for j in range(BD):
                g = slots[BD * s + j]
                t, lgi = divmod(g, BD)
                sel[s, t, K * j:K * (j + 1), K * lgi:K * (lgi + 1)] = eye
                used_set.add((s, t))
        sels.append(sel)
    used = tuple(sorted(used_set))

    in_maps = []
    for ci in range(N_CORES):
        a, b = bounds[ci], bounds[ci + 1]
        pc = b - a
        # partition-major x: [120, S*HW]; row 119 = ones (residual fold)
        rows = np.zeros((P_pad * K, HW), dtype=nbf16)
        if pc:
            rows[:pc * K] = xb[a:b].reshape(pc * K, HW)
        arr3 = np.zeros((S, BDK + 1, HW), dtype=nbf16)
        arr3[:, :BDK] = rows.reshape(S, BDK, HW)
        arr3[:, BDK] = 1.0
        xs = np.ascontiguousarray(
            arr3.transpose(1, 0, 2).reshape(BDK + 1, S * HW))
        sel = sels[ci]
        su = np.stack([sel[s, t] for (s, t) in used])  # [U, 119, 119]
        sel_pack = su.transpose(1, 0, 2).reshape(BDK, len(used) * BDK)
        selt_pack = su.transpose(2, 0, 1).reshape(BDK, len(used) * BDK)
        m = {
            "x": xs,
            "wq_col": wq_col,
            "wkt_bd": wkt_bd,
            "wv_aug": wv_aug,
            "id119": id119,
            "iaug": iaug,
            "bdmask": bdmask,
            "bdmaskb": bdmask.astype(nbf16),
            "sel": np.ascontiguousarray(sel_pack).astype(nbf16),
            "selT": np.ascontiguousarray(selt_pack).astype(nbf16),
        }
        if have_bias:
            corr_col = np.zeros((BDK, K * S), np.float32)
            if pc:
                cpad = np.zeros((P_pad, K, K), np.float32)
                cpad[:pc] = corr_all[a:b]
                for s in range(S):
                    for j in range(BD):
                        corr_col[K * j:K * (j + 1), K * s:K * (s + 1)] = \
                            cpad[BD * s + j]
            m["corr_col"] = corr_col
        in_maps.append(m)
    return in_maps, bounds, P_pad, T, have_bias, used


def _gather(results, bounds, P_pad):
    S = P_pad // BD
    out = np.empty((P_TOTAL, K, 64, 64), dtype=np.float32)
    for ci in range(N_CORES):
        a, b = bounds[ci], bounds[ci + 1]
        pc = b - a
        if pc:
            y = np.asarray(results[ci]["y"], dtype=np.float32)  # [119, S*HW]
            y = y.reshape(BDK, S, HW).transpose(1, 0, 2).reshape(
                P_pad, K, 64, 64)
            out[a:b] = y[:pc]
    return out


def _run(inputs: dict, trace: bool = False):
    _ensure_path()
    from concourse.bass_utils import run_bass_kernel_spmd

    in_maps, bounds, P_pad, T, have_bias, used = _prepare(inputs)
    nc = _get_compiled(P_pad, T, have_bias, used)
    res = run_bass_kernel_spmd(nc, in_maps, list(range(N_CORES)), trace=trace)
    return _gather(res.results, bounds, P_pad), res


def kernel(**inputs) -> np.ndarray:
    out, _ = _run(inputs, trace=False)
    return out
